# revision 1
# baseline (speedup 1.0000x reference)
"""Trainium2 Bass kernel for nn_PoissonNLLLoss (B=16, H=1024, W=2048, MAX_ID=356).

Computes  LOSS_WEIGHT * (mean(exp(logits)) - inst)  where inst is the mean over
images of the sum of logits gathered at per-segment centroids (segments are
label ids > 100), exactly matching the jax reference semantics.

Sharding: data-parallel over the batch — 2 images per NeuronCore across 8
cores (SPMD, identical program). Host combines the per-core partial scalars
(exp-sum and per-image instance sums), the only cross-core communication.

Per-core algorithm (all segment statistics are exact f32 integer arithmetic):
  id = 32*hi + lo. The image is processed in row-bands of 128 rows; within a
  band each column c is one "chunk" of 128 pixels (partition p = row in band).
  For each chunk one bf16 matmul accumulates into PSUM:
      psum[(s,j), i] += sum_p stat[p,(s,j)] * onehot_lo[p,i]
  with stationary stat = onehot_hi (x) {1, p, c mod 256} — all bf16-exact
  values — grouped per (band, 256-column octant). PSUM evacuations apply the
  exact f32 corrections  sy += 128*band*cnt_g  and  sx += 256*oct*cnt_g.
  One-hots are built on DVE/GPSIMD/ACT in transposed step-1 bf16 layouts
  (DVE 2x packed mode); exp+row-sum rides on ACT via accum_out.
  Finalize on device: centroids via exact floor division (reciprocal +/-1
  correction), indirect-DMA gather of logits at centroid offsets, validity
  masking, and partition reduction via a ones-matmul.
"""

import numpy as np

P = 128
NLO = 32
NHI = 12
NST = 3          # stationary stats {1, p, c mod 256}
MAX_ID = 356
NID = NLO * NHI  # 384 (ids >= 356 never occur -> cnt 0, masked)
OCT = 256        # column span of one PSUM accumulation group
NBLK = 5         # bounce blocks per image: cnt, Sp, Sc, corr_y, corr_x

B, H, W = 16, 1024, 2048
N_CORES = 8
NIMG = B // N_CORES


def _build_nc(n_img, H, W, G=128, trunc_cast=False):
    # trunc_cast: CoreSim truncates on f32->i32 copy; TRN2 HW rounds to
    # nearest. The hi-digit extraction bias must match the cast mode.
    cast_bias = 0.5 if trunc_cast else -15.5
    import concourse.bass as bass
    import concourse.bacc as bacc
    import concourse.tile as tile
    from concourse import mybir

    f32 = mybir.dt.float32
    i32 = mybir.dt.int32
    bf16 = mybir.dt.bfloat16
    Alu = mybir.AluOpType
    Act = mybir.ActivationFunctionType

    NB = H // P
    NOCT = max(W // OCT, 1)
    G = min(G, W)
    NBATCH = W // G
    BPO = max(NBATCH // NOCT, 1)
    M = NST * NHI
    n_btiles = n_img * NB

    nc = bacc.Bacc('TRN2', target_bir_lowering=False, debug=False)
    logits_h = nc.declare_dram_parameter("logits", [n_img, H, W], f32, isOutput=False)
    label_h = nc.declare_dram_parameter("label", [n_img, H, W], i32, isOutput=False)
    out_h = nc.declare_dram_parameter("out", [1, 4], f32, isOutput=True)
    bounce_h = nc.dram_tensor("bounce", [n_img * NBLK * NID], f32)

    with tile.TileContext(nc) as tc:
        import contextlib
        ctx = contextlib.ExitStack()
        with ctx:
            cpool = ctx.enter_context(tc.tile_pool(name="consts", bufs=1))
            bandA = ctx.enter_context(tc.tile_pool(name="bandA", bufs=3))
            bandB = ctx.enter_context(tc.tile_pool(name="bandB", bufs=3))
            batchp = ctx.enter_context(tc.tile_pool(name="batchp", bufs=4))
            accp = ctx.enter_context(tc.tile_pool(name="acc", bufs=1))
            psum = ctx.enter_context(tc.tile_pool(name="psum", bufs=4, space="PSUM"))
            fin = ctx.enter_context(tc.tile_pool(name="fin", bufs=1))

            # ---- constants (transposed step-1 bf16 layouts; values bf16-exact)
            iota32_t = cpool.tile([P, NLO * G], bf16)
            nc.gpsimd.iota(iota32_t[:].rearrange("p (i c) -> p i c", i=NLO),
                           pattern=[[1, NLO], [0, G]], base=0, channel_multiplier=0,
                           allow_small_or_imprecise_dtypes=True)
            iota12_t = cpool.tile([P, NHI * G], bf16)
            nc.gpsimd.iota(iota12_t[:].rearrange("p (j c) -> p j c", j=NHI),
                           pattern=[[1, NHI], [0, G]], base=0, channel_multiplier=0,
                           allow_small_or_imprecise_dtypes=True)
            OCTW = min(OCT, W)
            xr_t = cpool.tile([P, NHI * OCTW], bf16)
            nc.gpsimd.iota(xr_t[:].rearrange("p (j c) -> p j c", j=NHI),
                           pattern=[[0, NHI], [1, OCTW]], base=0, channel_multiplier=0,
                           allow_small_or_imprecise_dtypes=True)
            p_col = cpool.tile([P, 1], f32)
            nc.gpsimd.iota(p_col[:], pattern=[[0, 1]], base=0, channel_multiplier=1,
                           allow_small_or_imprecise_dtypes=True)
            ones_col = cpool.tile([P, 1], f32)
            nc.vector.memset(ones_col[:], 1.0)
            # id layout after bounce reload: id = 3*p + j at [p, img*3 + j]
            idf = cpool.tile([P, n_img * 3], f32)
            nc.gpsimd.iota(idf[:].rearrange("p (g i) -> p g i", g=n_img),
                           pattern=[[0, n_img], [1, 3]], base=0,
                           channel_multiplier=3,
                           allow_small_or_imprecise_dtypes=True)

            exp_accs = accp.tile([P, n_btiles], f32)

            accs = []
            for img in range(n_img):
                a = accp.tile([M, NLO], f32, tag=f"acc{img}")
                cy = accp.tile([NHI, NLO], f32, tag=f"accy{img}")
                cx = accp.tile([NHI, NLO], f32, tag=f"accx{img}")
                nc.vector.memset(a[:], 0.0)
                nc.vector.memset(cy[:], 0.0)
                nc.vector.memset(cx[:], 0.0)
                accs.append((a, cy, cx))

            for img in range(n_img):
                acc, acc2y, acc2x = accs[img]
                for band in range(NB):
                    r0 = band * P
                    label_band = bandA.tile([P, W], i32, tag="label_band")
                    nc.gpsimd.dma_start(out=label_band[:], in_=label_h[img, r0:r0 + P, :])
                    logits_band = bandA.tile([P, W], f32, tag="logits_band")
                    nc.gpsimd.dma_start(out=logits_band[:], in_=logits_h[img, r0:r0 + P, :])

                    # exp + per-partition row-sum fused on ACT
                    exp_scr = bandB.tile([P, W], f32, tag="exp_scr")
                    nc.scalar.activation(
                        out=exp_scr[:], in_=logits_band[:], func=Act.Exp,
                        accum_out=exp_accs[:, img * NB + band: img * NB + band + 1])

                    # hi = int_cast((label + cast_bias)/32); lo = label - 32*hi
                    hi_i = bandB.tile([P, W], i32, tag="hi_i")
                    nc.vector.tensor_scalar(out=hi_i[:], in0=label_band[:],
                                            scalar1=cast_bias, scalar2=1.0 / NLO,
                                            op0=Alu.add, op1=Alu.mult)
                    hi_bf = bandB.tile([P, W], bf16, tag="hi_bf")
                    nc.scalar.activation(out=hi_bf[:], in_=hi_i[:], func=Act.Copy)
                    lo_bf = bandB.tile([P, W], bf16, tag="lo_bf")
                    nc.vector.scalar_tensor_tensor(out=lo_bf[:], in0=hi_bf[:],
                                                   scalar=-float(NLO), in1=label_band[:],
                                                   op0=Alu.mult, op1=Alu.add)

                    for oct_i in range(NOCT):
                        ps = psum.tile([M, NLO], f32, tag="psband")
                        for bj in range(BPO):
                            bi = oct_i * BPO + bj
                            c0 = bi * G
                            # transposed layouts: innermost dim = chunk (step 1)
                            alo = batchp.tile([P, NLO * G], bf16, tag="alo")
                            alo_v = alo[:].rearrange("p (i c) -> p i c", i=NLO)
                            lo_b = lo_bf[:, c0:c0 + G].unsqueeze(1).to_broadcast([P, NLO, G])
                            nc.vector.tensor_tensor(
                                out=alo_v, in0=lo_b,
                                in1=iota32_t[:].rearrange("p (i c) -> p i c", i=NLO),
                                op=Alu.is_equal)

                            stat = batchp.tile([P, M * G], bf16, tag="stat")
                            stat_v = stat[:].rearrange("p (s j c) -> p s j c", s=NST, j=NHI)
                            hi_b = hi_bf[:, c0:c0 + G].unsqueeze(1).to_broadcast([P, NHI, G])
                            nc.vector.tensor_tensor(
                                out=stat_v[:, 0, :, :], in0=hi_b,
                                in1=iota12_t[:].rearrange("p (j c) -> p j c", j=NHI),
                                op=Alu.is_equal)
                            # stat * p on ACT (per-partition scale)
                            nc.scalar.activation(out=stat_v[:, 1, :, :],
                                                 in_=stat_v[:, 0, :, :],
                                                 func=Act.Copy, scale=p_col[:, 0:1])
                            # stat * (c mod 256) on GPSIMD
                            xr_sl = xr_t[:].rearrange("p (j c) -> p j c", j=NHI)[
                                :, :, bj * G:(bj + 1) * G]
                            nc.gpsimd.tensor_tensor(out=stat_v[:, 2, :, :],
                                                    in0=stat_v[:, 0, :, :],
                                                    in1=xr_sl, op=Alu.mult)

                            for g in range(G):
                                nc.tensor.matmul(
                                    out=ps[:],
                                    lhsT=stat_v[:, :, :, g],
                                    rhs=alo_v[:, :, g],
                                    start=(bj == 0 and g == 0),
                                    stop=(bj == BPO - 1 and g == G - 1),
                                )

                        # evacuate group: acc += ps; corr_y += 128*band*cnt_g;
                        # corr_x += 256*oct*cnt_g  (exact f32 for this data)
                        nc.vector.tensor_tensor(out=acc[:], in0=acc[:], in1=ps[:],
                                                op=Alu.add)
                        if band:
                            nc.vector.scalar_tensor_tensor(
                                out=acc2y[:], in0=ps[0:NHI, :], scalar=float(P * band),
                                in1=acc2y[:], op0=Alu.mult, op1=Alu.add)
                        if oct_i:
                            nc.vector.scalar_tensor_tensor(
                                out=acc2x[:], in0=ps[0:NHI, :], scalar=float(OCT * oct_i),
                                in1=acc2x[:], op0=Alu.mult, op1=Alu.add)

            # ---- finalize ----
            for img in range(n_img):
                acc, acc2y, acc2x = accs[img]
                base = img * NBLK * NID
                nc.gpsimd.dma_start(
                    out=bounce_h[base:base + 3 * NID].rearrange("(p c) -> p c", p=M),
                    in_=acc[:])
                nc.gpsimd.dma_start(
                    out=bounce_h[base + 3 * NID:base + 4 * NID]
                    .rearrange("(p c) -> p c", p=NHI), in_=acc2y[:])
                nc.gpsimd.dma_start(
                    out=bounce_h[base + 4 * NID:base + 5 * NID]
                    .rearrange("(p c) -> p c", p=NHI), in_=acc2x[:])

            def reload(s):
                t = fin.tile([P, n_img * 3], f32, tag=f"re{s}")
                src = bounce_h[:].rearrange("(i s p j) -> p i s j", i=n_img, s=NBLK, p=P)
                nc.gpsimd.dma_start(out=t[:].rearrange("p (i j) -> p i j", i=n_img),
                                    in_=src[:, :, s, :])
                return t

            cnt = reload(0)
            sy = reload(1)
            sx = reload(2)
            cry = reload(3)
            crx = reload(4)
            Alu_ = Alu
            nc.vector.tensor_tensor(out=sy[:], in0=sy[:], in1=cry[:], op=Alu_.add)
            nc.vector.tensor_tensor(out=sx[:], in0=sx[:], in1=crx[:], op=Alu_.add)

            denom = fin.tile([P, n_img * 3], f32, tag="denom")
            nc.vector.tensor_scalar(out=denom[:], in0=cnt[:], scalar1=1.0, scalar2=None,
                                    op0=Alu.max)
            rcp = fin.tile([P, n_img * 3], f32, tag="rcp")
            nc.vector.reciprocal(rcp[:], denom[:])

            def floordiv(s_t, nm):
                # exact floor(s/denom): approximate quotient then +/-1 fix,
                # insensitive to the f32->i32 cast rounding mode
                qf = fin.tile([P, n_img * 3], f32, tag=f"qf{nm}")
                nc.vector.tensor_tensor(out=qf[:], in0=s_t[:], in1=rcp[:], op=Alu.mult)
                qi = fin.tile([P, n_img * 3], i32, tag=f"qi{nm}")
                nc.vector.tensor_copy(qi[:], qf[:])
                q = fin.tile([P, n_img * 3], f32, tag=f"q{nm}")
                nc.vector.tensor_copy(q[:], qi[:])
                r = fin.tile([P, n_img * 3], f32, tag=f"r{nm}")
                nc.vector.tensor_tensor(out=r[:], in0=q[:], in1=denom[:], op=Alu.mult)
                nc.vector.tensor_tensor(out=r[:], in0=s_t[:], in1=r[:], op=Alu.subtract)
                corr = fin.tile([P, n_img * 3], f32, tag=f"corr{nm}")
                nc.vector.tensor_tensor(out=corr[:], in0=r[:], in1=denom[:], op=Alu.is_ge)
                nc.vector.tensor_tensor(out=q[:], in0=q[:], in1=corr[:], op=Alu.add)
                nc.vector.tensor_scalar(out=corr[:], in0=r[:], scalar1=0.0, scalar2=None,
                                        op0=Alu.is_lt)
                nc.vector.tensor_tensor(out=q[:], in0=q[:], in1=corr[:], op=Alu.subtract)
                return q

            qy = floordiv(sy, "y")
            qx = floordiv(sx, "x")

            offs_f = fin.tile([P, n_img * 3], f32, tag="offs_f")
            nc.vector.scalar_tensor_tensor(out=offs_f[:], in0=qy[:], scalar=float(W),
                                           in1=qx[:], op0=Alu.mult, op1=Alu.add)
            mask = fin.tile([P, n_img * 3], f32, tag="mask")
            nc.vector.tensor_scalar(out=mask[:], in0=idf[:], scalar1=100.0, scalar2=None,
                                    op0=Alu.is_gt)
            m2 = fin.tile([P, n_img * 3], f32, tag="m2")
            nc.vector.tensor_scalar(out=m2[:], in0=cnt[:], scalar1=0.0, scalar2=None,
                                    op0=Alu.is_gt)
            nc.vector.tensor_tensor(out=mask[:], in0=mask[:], in1=m2[:], op=Alu.mult)
            nc.vector.tensor_tensor(out=offs_f[:], in0=offs_f[:], in1=mask[:], op=Alu.mult)
            offs_i = fin.tile([P, n_img * 3], i32, tag="offs_i")
            nc.vector.tensor_copy(offs_i[:], offs_f[:])

            # gather logits at centroids (one offset per partition per DMA)
            gath = fin.tile([P, n_img * 3], f32, tag="gath")
            for img in range(n_img):
                for j in range(3):
                    col = img * 3 + j
                    nc.gpsimd.indirect_dma_start(
                        out=gath[:, col:col + 1],
                        out_offset=None,
                        in_=logits_h[:].rearrange("i h w -> (i h w)").unsqueeze(1),
                        in_offset=bass.IndirectOffsetOnAxis(
                            ap=offs_i[:, col:col + 1], axis=0),
                        element_offset=img * H * W,
                    )

            nc.vector.tensor_tensor(out=gath[:], in0=gath[:], in1=mask[:], op=Alu.mult)

            red = fin.tile([P, n_img + 1], f32, tag="red")
            for img in range(n_img):
                nc.vector.tensor_reduce(out=red[:, img:img + 1],
                                        in_=gath[:, img * 3:(img + 1) * 3],
                                        axis=mybir.AxisListType.X, op=Alu.add)
            nc.vector.tensor_reduce(out=red[:, n_img:n_img + 1], in_=exp_accs[:],
                                    axis=mybir.AxisListType.X, op=Alu.add)

            ps_fin = psum.tile([1, n_img + 1], f32, tag="ps_fin")
            nc.tensor.matmul(out=ps_fin[:], lhsT=ones_col[:], rhs=red[:],
                             start=True, stop=True)

            out_sb = fin.tile([1, 4], f32, tag="out_sb")
            nc.vector.memset(out_sb[:], 0.0)
            nc.vector.tensor_copy(out_sb[:, 0:1], ps_fin[:, n_img:n_img + 1])
            for img in range(n_img):
                nc.vector.tensor_copy(out_sb[:, 1 + img:2 + img], ps_fin[:, img:img + 1])
            nc.gpsimd.dma_start(out=out_h[:], in_=out_sb[:])

    nc.compile()
    return nc


_NC_CACHE = {}


def kernel(logits, label):
    logits = np.ascontiguousarray(np.asarray(logits, dtype=np.float32))
    label = np.ascontiguousarray(np.asarray(label, dtype=np.int32))
    assert logits.shape == (B, H, W), logits.shape
    assert label.shape == (B, H, W), label.shape

    from concourse.bass_utils import run_bass_kernel_spmd

    key = (NIMG, H, W)
    if key not in _NC_CACHE:
        _NC_CACHE[key] = _build_nc(NIMG, H, W, G=128)
    nc = _NC_CACHE[key]

    in_maps = [
        {"logits": logits[c * NIMG:(c + 1) * NIMG],
         "label": label[c * NIMG:(c + 1) * NIMG]}
        for c in range(N_CORES)
    ]
    # the axon-proxied device occasionally reports a transient
    # NRT_EXEC_UNIT_UNRECOVERABLE; retry a few times before giving up
    import time as _time
    last_exc = None
    for attempt in range(4):
        try:
            res = run_bass_kernel_spmd(nc, in_maps, list(range(N_CORES)))
            break
        except Exception as e:  # jax.errors.JaxRuntimeError and friends
            last_exc = e
            _time.sleep(2.0 * (attempt + 1))
    else:
        raise last_exc

    # host-side combine of the per-core partial scalars (the two "all-reduces")
    exp_total = 0.0
    inst_total = 0.0
    for c in range(N_CORES):
        o = res.results[c]["out"][0]
        exp_total += float(o[0])
        for i in range(NIMG):
            inst_total += float(o[1 + i])
    int_loss = exp_total / float(B * H * W)
    inst = inst_total / float(B)
    return np.float32(int_loss - inst)



# revision 2
# speedup vs baseline: 1.2591x; 1.2591x over previous
"""Trainium2 Bass kernel v2 for nn_PoissonNLLLoss (B=16, H=1024, W=2048).

Computes  mean(exp(logits)) - mean_img( sum_{id>100,cnt>0} logits[cy,cx] )
with exact integer segment statistics, matching the jax reference.

v2 design (vs baseline):
  - 256 bins via id' = label - 101  (valid ids 101..355 -> 0..254; invalid
    labels give negative id' whose hi-digit never matches -> self-masking).
  - digits hi = id' >> 4, lo = id' & 15 computed on int16 at DVE 4x.
  - one-hot planes built with fused tensor_scalar ops (4x DVE mode):
      stat[h]      = (hi == h)                  [is_equal]
      stat[16+h]   = (hi == h) * p              [is_equal , mult p-col AP]
      stat[32+h]   = (hi == h) * c              [tensor_tensor vs c-row, 2x]
      alo[l]       = (lo == l)                  [is_equal]
    c-plane values are the FULL column index (0..2047) — exact in fp16 —
    so there are no octant corrections and a single PSUM accumulator per
    band with a 3-op evacuation.
  - matmul lhsT = stat[:, :, j] [128, 48] fp16, rhs = alo[:, :, j]
    [128, 16] fp16 -> psum [48, 16] accumulated over the whole band.
  - finalize: bounce stats to DRAM, reload as [128, 12], exact floor
    division, one indirect-DMA gather per (img, half), masked sum.
"""

import numpy as np

P = 128
NH = 16          # hi digit values
NL = 16          # lo digit values
NSTAT = 3        # {1, p, c}
M = NSTAT * NH   # 48 stationary rows

B, H, W = 16, 1024, 2048
N_CORES = 8
NIMG = B // N_CORES


def _build_nc(n_img, H, W, G=512, csplit=8, act_p=-1, NH=16, NL=16):
    # csplit: how many of the NH c-planes go to GPSIMD (rest on DVE)
    # act_p: how many of the NH p-planes go to ACT (copy-scale; rest DVE fused)
    import concourse.bass as bass
    import concourse.bacc as bacc
    import concourse.tile as tile
    from concourse import mybir

    f32 = mybir.dt.float32
    i32 = mybir.dt.int32
    bf16 = mybir.dt.bfloat16
    f16 = mybir.dt.float16
    Alu = mybir.AluOpType
    Act = mybir.ActivationFunctionType

    NB = H // P              # bands per image (8)
    NCH = W // G             # chunks per band
    n_btiles = n_img * NB
    M = NSTAT * NH           # stationary rows
    assert NH * NL == 256

    nc = bacc.Bacc('TRN2', target_bir_lowering=False, debug=False)
    logits_h = nc.declare_dram_parameter("logits", [n_img, H, W], f32, isOutput=False)
    label_h = nc.declare_dram_parameter("label", [n_img, H, W], i32, isOutput=False)
    out_h = nc.declare_dram_parameter("out", [1, 4], f32, isOutput=True)
    # bounce: per image: cnt[256], sy_p[256], sx[256], ycoarse[256]
    bounce_h = nc.dram_tensor("bounce", [n_img * 4 * 256], f32)

    with tile.TileContext(nc) as tc:
        import contextlib
        ctx = contextlib.ExitStack()
        with ctx:
            cpool = ctx.enter_context(tc.tile_pool(name="consts", bufs=1))
            lab_pool = ctx.enter_context(tc.tile_pool(name="lab", bufs=2))
            log_pool = ctx.enter_context(tc.tile_pool(name="log", bufs=2))
            exp_pool = ctx.enter_context(tc.tile_pool(name="expp", bufs=1))
            idp_pool = ctx.enter_context(tc.tile_pool(name="idpp", bufs=1))
            dig_pool = ctx.enter_context(tc.tile_pool(name="dig", bufs=2))
            stat_pool = ctx.enter_context(tc.tile_pool(name="stat", bufs=2))
            alo_pool = ctx.enter_context(tc.tile_pool(name="alo", bufs=1))
            acc_pool = ctx.enter_context(tc.tile_pool(name="acc", bufs=1))
            psum = ctx.enter_context(tc.tile_pool(name="psum", bufs=4, space="PSUM"))
            fin = ctx.enter_context(tc.tile_pool(name="fin", bufs=1))

            # ---- constants
            p_col = cpool.tile([P, 1], f32)
            nc.gpsimd.iota(p_col[:], pattern=[[0, 1]], base=0, channel_multiplier=1,
                           allow_small_or_imprecise_dtypes=True)
            ones_col = cpool.tile([P, 1], f32)
            nc.vector.memset(ones_col[:], 1.0)
            # c-row: column index replicated across partitions, fp16 (0..2047)
            crow = cpool.tile([P, W], f16)
            nc.gpsimd.iota(crow[:], pattern=[[1, W]], base=0, channel_multiplier=0,
                           allow_small_or_imprecise_dtypes=True)
            # bin id per finalize layout: bin = 128*k + p at [p, (img, s, k)]
            # (only used for exp accum staging / masks built later)

            exp_accs = acc_pool.tile([P, n_btiles], f32)

            accs = []
            for img in range(n_img):
                a = acc_pool.tile([M, NL], f32, tag=f"acc{img}")
                ay = acc_pool.tile([NH, NL], f32, tag=f"accy{img}")
                nc.vector.memset(a[:], 0.0)
                nc.vector.memset(ay[:], 0.0)
                accs.append((a, ay))

            for img in range(n_img):
                acc, accY = accs[img]
                for band in range(NB):
                    r0 = band * P
                    bt = img * NB + band
                    label_band = lab_pool.tile([P, W], i32, tag="label_band")
                    nc.gpsimd.dma_start(out=label_band[:], in_=label_h[img, r0:r0 + P, :])
                    logits_band = log_pool.tile([P, W], f32, tag="logits_band")
                    nc.gpsimd.dma_start(out=logits_band[:], in_=logits_h[img, r0:r0 + P, :])

                    # exp + per-partition row-sum on ACT (scratch out, 1-buf)
                    exp_scr = exp_pool.tile([P, W], f32, tag="exp_scr")
                    nc.scalar.activation(
                        out=exp_scr[:], in_=logits_band[:], func=Act.Exp,
                        accum_out=exp_accs[:, bt:bt + 1])

                    # id' = label - 101 in bf16 (exact: |id'| <= 256); invalid
                    # labels give negative id' -> negative hi -> no plane fires
                    idp_bf = idp_pool.tile([P, W], bf16, tag="idp")
                    nc.scalar.activation(out=idp_bf[:], in_=label_band[:],
                                         func=Act.Copy, bias=-101.0)
                    # hi = floor(id'/NL) via RNE(label*(1/NL) + bias) on ACT -> i32
                    hi_i = idp_pool.tile([P, W], i32, tag="hi_i32")
                    nc.scalar.activation(out=hi_i[:], in_=label_band[:],
                                         func=Act.Copy, scale=1.0 / NL,
                                         bias=(-101.0 - (NL - 1) / 2.0) / NL)
                    hi_bf = dig_pool.tile([P, W], bf16, tag="hi_bf")
                    nc.scalar.activation(out=hi_bf[:], in_=hi_i[:], func=Act.Copy)
                    # lo = id' - NL*hi (bf16 exact)
                    lo_bf = dig_pool.tile([P, W], bf16, tag="lo_bf")
                    nc.vector.scalar_tensor_tensor(out=lo_bf[:], in0=hi_bf[:],
                                                   scalar=-float(NL), in1=idp_bf[:],
                                                   op0=Alu.mult, op1=Alu.add)

                    ps = psum.tile([M, NL], f32, tag="ps")
                    for ch in range(NCH):
                        c0 = ch * G
                        stat = stat_pool.tile([P, M * G], f16, tag="stat")
                        stat_v = stat[:].rearrange("p (m c) -> p m c", m=M)
                        alo = alo_pool.tile([P, NL * G], f16, tag="alo")
                        alo_v = alo[:].rearrange("p (l c) -> p l c", l=NL)

                        for h in range(NH):
                            # hi-onehot
                            nc.vector.tensor_scalar(
                                out=stat_v[:, h, :], in0=hi_bf[:, c0:c0 + G],
                                scalar1=float(h), scalar2=None, op0=Alu.is_equal)
                        # p * hi-onehot: first nb planes as one bulk ACT
                        # copy-scale, the rest fused on DVE
                        nb = NH if act_p < 0 else act_p
                        if nb > 0:
                            nc.scalar.activation(
                                out=stat_v[:, NH:NH + nb, :],
                                in_=stat_v[:, 0:nb, :], func=Act.Copy,
                                scale=p_col[:, 0:1])
                        for h in range(nb, NH):
                            nc.vector.tensor_scalar(
                                out=stat_v[:, NH + h, :],
                                in0=hi_bf[:, c0:c0 + G],
                                scalar1=float(h), scalar2=p_col[:, 0:1],
                                op0=Alu.is_equal, op1=Alu.mult)
                        # c * hi-onehot via tensor_tensor against c-row
                        for h in range(NH):
                            eng = nc.gpsimd if h < csplit else nc.vector
                            eng.tensor_tensor(
                                out=stat_v[:, 2 * NH + h, :],
                                in0=stat_v[:, h, :],
                                in1=crow[:, c0:c0 + G],
                                op=Alu.mult)
                        for l in range(NL):
                            nc.vector.tensor_scalar(
                                out=alo_v[:, l, :], in0=lo_bf[:, c0:c0 + G],
                                scalar1=float(l), scalar2=None, op0=Alu.is_equal)

                        for g in range(G):
                            nc.tensor.matmul(
                                out=ps[:],
                                lhsT=stat_v[:, :, g],
                                rhs=alo_v[:, :, g],
                                start=(ch == 0 and g == 0),
                                stop=(ch == NCH - 1 and g == G - 1),
                            )

                    # evacuate band: acc += ps; accY += 128*band*cnt
                    nc.vector.tensor_tensor(out=acc[:], in0=acc[:], in1=ps[:],
                                            op=Alu.add)
                    if band:
                        nc.vector.scalar_tensor_tensor(
                            out=accY[:], in0=ps[0:NH, :], scalar=float(P * band),
                            in1=accY[:], op0=Alu.mult, op1=Alu.add)

            # ---- bounce stats to DRAM and reload in finalize layout ----
            # bounce flat layout: img*1024 + s*256 + bin   (bin = 16*h + l)
            for img in range(n_img):
                acc, accY = accs[img]
                base = img * 4 * 256
                nc.gpsimd.dma_start(
                    out=bounce_h[base:base + 3 * 256].rearrange("(p c) -> p c", p=M),
                    in_=acc[:])
                nc.gpsimd.dma_start(
                    out=bounce_h[base + 3 * 256:base + 4 * 256]
                    .rearrange("(p c) -> p c", p=NH), in_=accY[:])

            # reload: t[p, (img, s2, k)] = bounce[img*1024 + s*256 + 128k + p]
            def reload(s):
                t = fin.tile([P, n_img * 2], f32, tag=f"re{s}")
                src = bounce_h[:].rearrange("(i s k p) -> p i s k", i=n_img, s=4, k=2)
                for img in range(n_img):
                    nc.gpsimd.dma_start(out=t[:, img * 2:(img + 1) * 2],
                                        in_=src[:, img, s, :])
                return t

            cnt = reload(0)      # counts
            syp = reload(1)      # sum of p (y-fine)
            sx = reload(2)       # sum of c (exact full x)
            ycrs = reload(3)     # 128*band-weighted counts (y-coarse)
            sy = fin.tile([P, n_img * 2], f32, tag="sy")
            nc.vector.tensor_tensor(out=sy[:], in0=syp[:], in1=ycrs[:], op=mybir.AluOpType.add)

            denom = fin.tile([P, n_img * 2], f32, tag="denom")
            nc.vector.tensor_scalar(out=denom[:], in0=cnt[:], scalar1=1.0, scalar2=None,
                                    op0=Alu.max)
            rcp = fin.tile([P, n_img * 2], f32, tag="rcp")
            nc.vector.reciprocal(rcp[:], denom[:])

            def floordiv(s_t, nm):
                # exact floor(s/denom): approx quotient then +/-1 fix
                qf = fin.tile([P, n_img * 2], f32, tag=f"qf{nm}")
                nc.vector.tensor_tensor(out=qf[:], in0=s_t[:], in1=rcp[:], op=Alu.mult)
                qi = fin.tile([P, n_img * 2], i32, tag=f"qi{nm}")
                nc.vector.tensor_copy(qi[:], qf[:])
                q = fin.tile([P, n_img * 2], f32, tag=f"q{nm}")
                nc.vector.tensor_copy(q[:], qi[:])
                r = fin.tile([P, n_img * 2], f32, tag=f"r{nm}")
                nc.vector.tensor_tensor(out=r[:], in0=q[:], in1=denom[:], op=Alu.mult)
                nc.vector.tensor_tensor(out=r[:], in0=s_t[:], in1=r[:], op=Alu.subtract)
                corr = fin.tile([P, n_img * 2], f32, tag=f"corr{nm}")
                nc.vector.tensor_tensor(out=corr[:], in0=r[:], in1=denom[:], op=Alu.is_ge)
                nc.vector.tensor_tensor(out=q[:], in0=q[:], in1=corr[:], op=Alu.add)
                nc.vector.tensor_scalar(out=corr[:], in0=r[:], scalar1=0.0, scalar2=None,
                                        op0=Alu.is_lt)
                nc.vector.tensor_tensor(out=q[:], in0=q[:], in1=corr[:], op=Alu.subtract)
                return q

            qy = floordiv(sy, "y")
            qx = floordiv(sx, "x")

            offs_f = fin.tile([P, n_img * 2], f32, tag="offs_f")
            nc.vector.scalar_tensor_tensor(out=offs_f[:], in0=qy[:], scalar=float(W),
                                           in1=qx[:], op0=Alu.mult, op1=Alu.add)
            # all bins are ids > 100; only mask = cnt > 0 (and bin 255 unused,
            # its cnt is 0). Also zero offsets for masked bins (safe gather).
            mask = fin.tile([P, n_img * 2], f32, tag="mask")
            nc.vector.tensor_scalar(out=mask[:], in0=cnt[:], scalar1=0.0, scalar2=None,
                                    op0=Alu.is_gt)
            nc.vector.tensor_tensor(out=offs_f[:], in0=offs_f[:], in1=mask[:], op=Alu.mult)
            offs_i = fin.tile([P, n_img * 2], i32, tag="offs_i")
            nc.vector.tensor_copy(offs_i[:], offs_f[:])

            gath = fin.tile([P, n_img * 2], f32, tag="gath")
            for img in range(n_img):
                for k in range(2):
                    col = img * 2 + k
                    nc.gpsimd.indirect_dma_start(
                        out=gath[:, col:col + 1],
                        out_offset=None,
                        in_=logits_h[:].rearrange("i h w -> (i h w)").unsqueeze(1),
                        in_offset=bass.IndirectOffsetOnAxis(
                            ap=offs_i[:, col:col + 1], axis=0),
                        element_offset=img * H * W,
                    )

            nc.vector.tensor_tensor(out=gath[:], in0=gath[:], in1=mask[:], op=Alu.mult)

            red = fin.tile([P, n_img + 1], f32, tag="red")
            for img in range(n_img):
                nc.vector.tensor_reduce(out=red[:, img:img + 1],
                                        in_=gath[:, img * 2:(img + 1) * 2],
                                        axis=mybir.AxisListType.X, op=Alu.add)
            nc.vector.tensor_reduce(out=red[:, n_img:n_img + 1], in_=exp_accs[:],
                                    axis=mybir.AxisListType.X, op=Alu.add)

            ps_fin = psum.tile([1, n_img + 1], f32, tag="ps_fin")
            nc.tensor.matmul(out=ps_fin[:], lhsT=ones_col[:], rhs=red[:],
                             start=True, stop=True)

            out_sb = fin.tile([1, 4], f32, tag="out_sb")
            nc.vector.memset(out_sb[:], 0.0)
            nc.vector.tensor_copy(out_sb[:, 0:1], ps_fin[:, n_img:n_img + 1])
            for img in range(n_img):
                nc.vector.tensor_copy(out_sb[:, 1 + img:2 + img], ps_fin[:, img:img + 1])
            nc.gpsimd.dma_start(out=out_h[:], in_=out_sb[:])

    nc.compile()
    return nc


_NC_CACHE = {}


def kernel(logits, label):
    logits = np.ascontiguousarray(np.asarray(logits, dtype=np.float32))
    label = np.ascontiguousarray(np.asarray(label, dtype=np.int32))
    assert logits.shape == (B, H, W), logits.shape
    assert label.shape == (B, H, W), label.shape

    from concourse.bass_utils import run_bass_kernel_spmd

    key = (NIMG, H, W)
    if key not in _NC_CACHE:
        _NC_CACHE[key] = _build_nc(NIMG, H, W)
    nc = _NC_CACHE[key]

    in_maps = [
        {"logits": logits[c * NIMG:(c + 1) * NIMG],
         "label": label[c * NIMG:(c + 1) * NIMG]}
        for c in range(N_CORES)
    ]
    import time as _time
    last_exc = None
    for attempt in range(4):
        try:
            res = run_bass_kernel_spmd(nc, in_maps, list(range(N_CORES)))
            break
        except Exception as e:
            last_exc = e
            _time.sleep(2.0 * (attempt + 1))
    else:
        raise last_exc

    exp_total = 0.0
    inst_total = 0.0
    for c in range(N_CORES):
        o = res.results[c]["out"][0]
        exp_total += float(o[0])
        for i in range(NIMG):
            inst_total += float(o[1 + i])
    int_loss = exp_total / float(B * H * W)
    inst = inst_total / float(B)
    return np.float32(int_loss - inst)


# revision 3
# speedup vs baseline: 1.2680x; 1.0070x over previous
"""Trainium2 Bass kernel v2 for nn_PoissonNLLLoss (B=16, H=1024, W=2048).

Computes  mean(exp(logits)) - mean_img( sum_{id>100,cnt>0} logits[cy,cx] )
with exact integer segment statistics, matching the jax reference.

v2 design (vs baseline):
  - 256 bins via id' = label - 101  (valid ids 101..355 -> 0..254; invalid
    labels give negative id' whose hi-digit never matches -> self-masking).
  - digits hi = id' >> 4, lo = id' & 15 computed on int16 at DVE 4x.
  - one-hot planes built with fused tensor_scalar ops (4x DVE mode):
      stat[h]      = (hi == h)                  [is_equal]
      stat[16+h]   = (hi == h) * p              [is_equal , mult p-col AP]
      stat[32+h]   = (hi == h) * c              [tensor_tensor vs c-row, 2x]
      alo[l]       = (lo == l)                  [is_equal]
    c-plane values are the FULL column index (0..2047) — exact in fp16 —
    so there are no octant corrections and a single PSUM accumulator per
    band with a 3-op evacuation.
  - matmul lhsT = stat[:, :, j] [128, 48] fp16, rhs = alo[:, :, j]
    [128, 16] fp16 -> psum [48, 16] accumulated over the whole band.
  - finalize: bounce stats to DRAM, reload as [128, 12], exact floor
    division, one indirect-DMA gather per (img, half), masked sum.
"""

import numpy as np

P = 128
NH = 16          # hi digit values
NL = 16          # lo digit values
NSTAT = 3        # {1, p, c}
M = NSTAT * NH   # 48 stationary rows

B, H, W = 16, 1024, 2048
N_CORES = 8
NIMG = B // N_CORES


def _build_nc(n_img, H, W, G=512, csplit=8, act_p=-1, NH=16, NL=16):
    # csplit: how many of the NH c-planes go to GPSIMD (rest on DVE)
    # act_p: how many of the NH p-planes go to ACT (copy-scale; rest DVE fused)
    import concourse.bass as bass
    import concourse.bacc as bacc
    import concourse.tile as tile
    from concourse import mybir

    f32 = mybir.dt.float32
    i32 = mybir.dt.int32
    bf16 = mybir.dt.bfloat16
    f16 = mybir.dt.float16
    Alu = mybir.AluOpType
    Act = mybir.ActivationFunctionType

    NB = H // P              # bands per image (8)
    NCH = W // G             # chunks per band
    n_btiles = n_img * NB
    M = NSTAT * NH           # stationary rows
    assert NH * NL == 256

    nc = bacc.Bacc('TRN2', target_bir_lowering=False, debug=False)
    logits_h = nc.declare_dram_parameter("logits", [n_img, H, W], f32, isOutput=False)
    label_h = nc.declare_dram_parameter("label", [n_img, H, W], i32, isOutput=False)
    out_h = nc.declare_dram_parameter("out", [1, 4], f32, isOutput=True)
    # bounce: per image: cnt[256], sy_p[256], sx[256], ycoarse[256]
    bounce_h = nc.dram_tensor("bounce", [n_img * 4 * 256], f32)

    with tile.TileContext(nc) as tc:
        import contextlib
        ctx = contextlib.ExitStack()
        with ctx:
            cpool = ctx.enter_context(tc.tile_pool(name="consts", bufs=1))
            lab_pool = ctx.enter_context(tc.tile_pool(name="lab", bufs=2))
            log_pool = ctx.enter_context(tc.tile_pool(name="log", bufs=2))
            exp_pool = ctx.enter_context(tc.tile_pool(name="expp", bufs=1))
            idp_pool = ctx.enter_context(tc.tile_pool(name="idpp", bufs=1))
            dig_pool = ctx.enter_context(tc.tile_pool(name="dig", bufs=2))
            stat_pool = ctx.enter_context(tc.tile_pool(name="stat", bufs=2))
            alo_pool = ctx.enter_context(tc.tile_pool(name="alo", bufs=1))
            acc_pool = ctx.enter_context(tc.tile_pool(name="acc", bufs=1))
            psum = ctx.enter_context(tc.tile_pool(name="psum", bufs=4, space="PSUM"))
            fin = ctx.enter_context(tc.tile_pool(name="fin", bufs=1))

            # ---- constants
            p_col = cpool.tile([P, 1], f32)
            nc.gpsimd.iota(p_col[:], pattern=[[0, 1]], base=0, channel_multiplier=1,
                           allow_small_or_imprecise_dtypes=True)
            ones_col = cpool.tile([P, 1], f32)
            nc.vector.memset(ones_col[:], 1.0)
            # c-row: column index replicated across partitions, fp16 (0..2047)
            crow = cpool.tile([P, W], f16)
            nc.gpsimd.iota(crow[:], pattern=[[1, W]], base=0, channel_multiplier=0,
                           allow_small_or_imprecise_dtypes=True)
            # bin id per finalize layout: bin = 128*k + p at [p, (img, s, k)]
            # (only used for exp accum staging / masks built later)

            exp_accs = acc_pool.tile([P, n_btiles], f32)

            accs = []
            for img in range(n_img):
                a = acc_pool.tile([M, NL], f32, tag=f"acc{img}")
                ay = acc_pool.tile([NH, NL], f32, tag=f"accy{img}")
                nc.vector.memset(a[:], 0.0)
                nc.vector.memset(ay[:], 0.0)
                accs.append((a, ay))

            for img in range(n_img):
                acc, accY = accs[img]
                for band in range(NB):
                    r0 = band * P
                    bt = img * NB + band
                    label_band = lab_pool.tile([P, W], i32, tag="label_band")
                    nc.gpsimd.dma_start(out=label_band[:], in_=label_h[img, r0:r0 + P, :])
                    logits_band = log_pool.tile([P, W], f32, tag="logits_band")
                    nc.gpsimd.dma_start(out=logits_band[:], in_=logits_h[img, r0:r0 + P, :])

                    # exp + per-partition row-sum on ACT (scratch out, 1-buf)
                    exp_scr = exp_pool.tile([P, W], f32, tag="exp_scr")
                    nc.scalar.activation(
                        out=exp_scr[:], in_=logits_band[:], func=Act.Exp,
                        accum_out=exp_accs[:, bt:bt + 1])

                    # id' = label - 101 in bf16 (exact: |id'| <= 256); invalid
                    # labels give negative id' -> negative hi -> no plane fires
                    idp_bf = idp_pool.tile([P, W], bf16, tag="idp")
                    nc.scalar.activation(out=idp_bf[:], in_=label_band[:],
                                         func=Act.Copy, bias=-101.0)
                    # hi = floor(id'/NL) via RNE(label*(1/NL) + bias) on ACT -> i32
                    hi_i = idp_pool.tile([P, W], i32, tag="hi_i32")
                    nc.vector.tensor_scalar(out=hi_i[:], in0=label_band[:],
                                            scalar1=-101.0 - (NL - 1) / 2.0,
                                            scalar2=1.0 / NL,
                                            op0=Alu.add, op1=Alu.mult)
                    hi_bf = dig_pool.tile([P, W], bf16, tag="hi_bf")
                    nc.scalar.activation(out=hi_bf[:], in_=hi_i[:], func=Act.Copy)
                    # lo = id' - NL*hi (bf16 exact)
                    lo_bf = dig_pool.tile([P, W], bf16, tag="lo_bf")
                    nc.vector.scalar_tensor_tensor(out=lo_bf[:], in0=hi_bf[:],
                                                   scalar=-float(NL), in1=idp_bf[:],
                                                   op0=Alu.mult, op1=Alu.add)

                    ps = psum.tile([M, NL], f32, tag="ps")
                    for ch in range(NCH):
                        c0 = ch * G
                        stat = stat_pool.tile([P, M * G], f16, tag="stat")
                        stat_v = stat[:].rearrange("p (m c) -> p m c", m=M)
                        alo = alo_pool.tile([P, NL * G], f16, tag="alo")
                        alo_v = alo[:].rearrange("p (l c) -> p l c", l=NL)

                        for h in range(NH):
                            # hi-onehot
                            nc.vector.tensor_scalar(
                                out=stat_v[:, h, :], in0=hi_bf[:, c0:c0 + G],
                                scalar1=float(h), scalar2=None, op0=Alu.is_equal)
                        # p * hi-onehot: first nb planes as one bulk ACT
                        # copy-scale, the rest fused on DVE
                        nb = NH if act_p < 0 else act_p
                        if nb > 0:
                            nc.scalar.activation(
                                out=stat_v[:, NH:NH + nb, :],
                                in_=stat_v[:, 0:nb, :], func=Act.Copy,
                                scale=p_col[:, 0:1])
                        for h in range(nb, NH):
                            nc.vector.tensor_scalar(
                                out=stat_v[:, NH + h, :],
                                in0=hi_bf[:, c0:c0 + G],
                                scalar1=float(h), scalar2=p_col[:, 0:1],
                                op0=Alu.is_equal, op1=Alu.mult)
                        # c * hi-onehot via tensor_tensor against c-row
                        for h in range(NH):
                            eng = nc.gpsimd if h < csplit else nc.vector
                            eng.tensor_tensor(
                                out=stat_v[:, 2 * NH + h, :],
                                in0=stat_v[:, h, :],
                                in1=crow[:, c0:c0 + G],
                                op=Alu.mult)
                        for l in range(NL):
                            nc.vector.tensor_scalar(
                                out=alo_v[:, l, :], in0=lo_bf[:, c0:c0 + G],
                                scalar1=float(l), scalar2=None, op0=Alu.is_equal)

                        for g in range(G):
                            nc.tensor.matmul(
                                out=ps[:],
                                lhsT=stat_v[:, :, g],
                                rhs=alo_v[:, :, g],
                                start=(ch == 0 and g == 0),
                                stop=(ch == NCH - 1 and g == G - 1),
                            )

                    # evacuate band: acc += ps; accY += 128*band*cnt
                    nc.vector.tensor_tensor(out=acc[:], in0=acc[:], in1=ps[:],
                                            op=Alu.add)
                    if band:
                        nc.vector.scalar_tensor_tensor(
                            out=accY[:], in0=ps[0:NH, :], scalar=float(P * band),
                            in1=accY[:], op0=Alu.mult, op1=Alu.add)

            # ---- bounce stats to DRAM and reload in finalize layout ----
            # bounce flat layout: img*1024 + s*256 + bin   (bin = 16*h + l)
            for img in range(n_img):
                acc, accY = accs[img]
                base = img * 4 * 256
                nc.gpsimd.dma_start(
                    out=bounce_h[base:base + 3 * 256].rearrange("(p c) -> p c", p=M),
                    in_=acc[:])
                nc.gpsimd.dma_start(
                    out=bounce_h[base + 3 * 256:base + 4 * 256]
                    .rearrange("(p c) -> p c", p=NH), in_=accY[:])

            # reload: t[p, (img, s2, k)] = bounce[img*1024 + s*256 + 128k + p]
            def reload(s):
                t = fin.tile([P, n_img * 2], f32, tag=f"re{s}")
                src = bounce_h[:].rearrange("(i s k p) -> p i s k", i=n_img, s=4, k=2)
                for img in range(n_img):
                    nc.gpsimd.dma_start(out=t[:, img * 2:(img + 1) * 2],
                                        in_=src[:, img, s, :])
                return t

            cnt = reload(0)      # counts
            syp = reload(1)      # sum of p (y-fine)
            sx = reload(2)       # sum of c (exact full x)
            ycrs = reload(3)     # 128*band-weighted counts (y-coarse)
            sy = fin.tile([P, n_img * 2], f32, tag="sy")
            nc.vector.tensor_tensor(out=sy[:], in0=syp[:], in1=ycrs[:], op=mybir.AluOpType.add)

            denom = fin.tile([P, n_img * 2], f32, tag="denom")
            nc.vector.tensor_scalar(out=denom[:], in0=cnt[:], scalar1=1.0, scalar2=None,
                                    op0=Alu.max)
            rcp = fin.tile([P, n_img * 2], f32, tag="rcp")
            nc.vector.reciprocal(rcp[:], denom[:])

            def floordiv(s_t, nm):
                # exact floor(s/denom): approx quotient then +/-1 fix
                qf = fin.tile([P, n_img * 2], f32, tag=f"qf{nm}")
                nc.vector.tensor_tensor(out=qf[:], in0=s_t[:], in1=rcp[:], op=Alu.mult)
                qi = fin.tile([P, n_img * 2], i32, tag=f"qi{nm}")
                nc.vector.tensor_copy(qi[:], qf[:])
                q = fin.tile([P, n_img * 2], f32, tag=f"q{nm}")
                nc.vector.tensor_copy(q[:], qi[:])
                r = fin.tile([P, n_img * 2], f32, tag=f"r{nm}")
                nc.vector.tensor_tensor(out=r[:], in0=q[:], in1=denom[:], op=Alu.mult)
                nc.vector.tensor_tensor(out=r[:], in0=s_t[:], in1=r[:], op=Alu.subtract)
                corr = fin.tile([P, n_img * 2], f32, tag=f"corr{nm}")
                nc.vector.tensor_tensor(out=corr[:], in0=r[:], in1=denom[:], op=Alu.is_ge)
                nc.vector.tensor_tensor(out=q[:], in0=q[:], in1=corr[:], op=Alu.add)
                nc.vector.tensor_scalar(out=corr[:], in0=r[:], scalar1=0.0, scalar2=None,
                                        op0=Alu.is_lt)
                nc.vector.tensor_tensor(out=q[:], in0=q[:], in1=corr[:], op=Alu.subtract)
                return q

            qy = floordiv(sy, "y")
            qx = floordiv(sx, "x")

            offs_f = fin.tile([P, n_img * 2], f32, tag="offs_f")
            nc.vector.scalar_tensor_tensor(out=offs_f[:], in0=qy[:], scalar=float(W),
                                           in1=qx[:], op0=Alu.mult, op1=Alu.add)
            # all bins are ids > 100; only mask = cnt > 0 (and bin 255 unused,
            # its cnt is 0). Also zero offsets for masked bins (safe gather).
            mask = fin.tile([P, n_img * 2], f32, tag="mask")
            nc.vector.tensor_scalar(out=mask[:], in0=cnt[:], scalar1=0.0, scalar2=None,
                                    op0=Alu.is_gt)
            nc.vector.tensor_tensor(out=offs_f[:], in0=offs_f[:], in1=mask[:], op=Alu.mult)
            offs_i = fin.tile([P, n_img * 2], i32, tag="offs_i")
            nc.vector.tensor_copy(offs_i[:], offs_f[:])

            gath = fin.tile([P, n_img * 2], f32, tag="gath")
            for img in range(n_img):
                for k in range(2):
                    col = img * 2 + k
                    nc.gpsimd.indirect_dma_start(
                        out=gath[:, col:col + 1],
                        out_offset=None,
                        in_=logits_h[:].rearrange("i h w -> (i h w)").unsqueeze(1),
                        in_offset=bass.IndirectOffsetOnAxis(
                            ap=offs_i[:, col:col + 1], axis=0),
                        element_offset=img * H * W,
                    )

            nc.vector.tensor_tensor(out=gath[:], in0=gath[:], in1=mask[:], op=Alu.mult)

            red = fin.tile([P, n_img + 1], f32, tag="red")
            for img in range(n_img):
                nc.vector.tensor_reduce(out=red[:, img:img + 1],
                                        in_=gath[:, img * 2:(img + 1) * 2],
                                        axis=mybir.AxisListType.X, op=Alu.add)
            nc.vector.tensor_reduce(out=red[:, n_img:n_img + 1], in_=exp_accs[:],
                                    axis=mybir.AxisListType.X, op=Alu.add)

            ps_fin = psum.tile([1, n_img + 1], f32, tag="ps_fin")
            nc.tensor.matmul(out=ps_fin[:], lhsT=ones_col[:], rhs=red[:],
                             start=True, stop=True)

            out_sb = fin.tile([1, 4], f32, tag="out_sb")
            nc.vector.memset(out_sb[:], 0.0)
            nc.vector.tensor_copy(out_sb[:, 0:1], ps_fin[:, n_img:n_img + 1])
            for img in range(n_img):
                nc.vector.tensor_copy(out_sb[:, 1 + img:2 + img], ps_fin[:, img:img + 1])
            nc.gpsimd.dma_start(out=out_h[:], in_=out_sb[:])

    nc.compile()
    return nc


_NC_CACHE = {}


def kernel(logits, label):
    logits = np.ascontiguousarray(np.asarray(logits, dtype=np.float32))
    label = np.ascontiguousarray(np.asarray(label, dtype=np.int32))
    assert logits.shape == (B, H, W), logits.shape
    assert label.shape == (B, H, W), label.shape

    from concourse.bass_utils import run_bass_kernel_spmd

    key = (NIMG, H, W)
    if key not in _NC_CACHE:
        _NC_CACHE[key] = _build_nc(NIMG, H, W)
    nc = _NC_CACHE[key]

    in_maps = [
        {"logits": logits[c * NIMG:(c + 1) * NIMG],
         "label": label[c * NIMG:(c + 1) * NIMG]}
        for c in range(N_CORES)
    ]
    import time as _time
    last_exc = None
    for attempt in range(4):
        try:
            res = run_bass_kernel_spmd(nc, in_maps, list(range(N_CORES)))
            break
        except Exception as e:
            last_exc = e
            _time.sleep(2.0 * (attempt + 1))
    else:
        raise last_exc

    exp_total = 0.0
    inst_total = 0.0
    for c in range(N_CORES):
        o = res.results[c]["out"][0]
        exp_total += float(o[0])
        for i in range(NIMG):
            inst_total += float(o[1 + i])
    int_loss = exp_total / float(B * H * W)
    inst = inst_total / float(B)
    return np.float32(int_loss - inst)


# revision 5
# speedup vs baseline: 1.2698x; 1.0014x over previous
"""Trainium2 Bass kernel v2 for nn_PoissonNLLLoss (B=16, H=1024, W=2048).

Computes  mean(exp(logits)) - mean_img( sum_{id>100,cnt>0} logits[cy,cx] )
with exact integer segment statistics, matching the jax reference.

v2 design (vs baseline):
  - 256 bins via id' = label - 101  (valid ids 101..355 -> 0..254; invalid
    labels give negative id' whose hi-digit never matches -> self-masking).
  - digits: hi = floor(id'/16) via an exact round-to-nearest trick on DVE
    (i32), cast to bf16 on ACT; lo = id' - 16*hi in bf16 (all values exact).
  - one-hot planes built with fused tensor_scalar ops (4x DVE mode):
      stat[h]      = (hi == h)                  [is_equal]
      stat[16+h]   = (hi == h) * p              [is_equal , mult p-col AP]
      stat[32+h]   = (hi == h) * c              [tensor_tensor vs c-row, 2x]
      alo[l]       = (lo == l)                  [is_equal]
    c-plane values are the FULL column index (0..2047) — exact in fp16 —
    so there are no octant corrections and a single PSUM accumulator per
    band with a 3-op evacuation.
  - matmul lhsT = stat[:, :, j] [128, 48] fp16, rhs = alo[:, :, j]
    [128, 16] fp16 -> psum [48, 16] accumulated over the whole band.
  - finalize: bounce stats to DRAM, reload as [128, 12], exact floor
    division, one indirect-DMA gather per (img, half), masked sum.
"""

import numpy as np

P = 128
NH = 16          # hi digit values
NL = 16          # lo digit values
NSTAT = 3        # {1, p, c}
M = NSTAT * NH   # 48 stationary rows

B, H, W = 16, 1024, 2048
N_CORES = 8
NIMG = B // N_CORES


def _build_nc(n_img, H, W, G=512, csplit=8, act_p=-1, NH=16, NL=16):
    # csplit: how many of the NH c-planes go to GPSIMD (rest on DVE)
    # act_p: how many of the NH p-planes go to ACT (copy-scale; rest DVE fused)
    import concourse.bass as bass
    import concourse.bacc as bacc
    import concourse.tile as tile
    from concourse import mybir

    f32 = mybir.dt.float32
    i32 = mybir.dt.int32
    bf16 = mybir.dt.bfloat16
    f16 = mybir.dt.float16
    Alu = mybir.AluOpType
    Act = mybir.ActivationFunctionType

    NB = H // P              # bands per image (8)
    NCH = W // G             # chunks per band
    n_btiles = n_img * NB
    M = NSTAT * NH           # stationary rows
    assert NH * NL == 256

    nc = bacc.Bacc('TRN2', target_bir_lowering=False, debug=False)
    logits_h = nc.declare_dram_parameter("logits", [n_img, H, W], f32, isOutput=False)
    label_h = nc.declare_dram_parameter("label", [n_img, H, W], i32, isOutput=False)
    out_h = nc.declare_dram_parameter("out", [1, 4], f32, isOutput=True)
    # bounce: per image: cnt[256], sy_p[256], sx[256], ycoarse[256]
    bounce_h = nc.dram_tensor("bounce", [n_img * 4 * 256], f32)

    with tile.TileContext(nc) as tc:
        import contextlib
        ctx = contextlib.ExitStack()
        with ctx:
            cpool = ctx.enter_context(tc.tile_pool(name="consts", bufs=1))
            lab_pool = ctx.enter_context(tc.tile_pool(name="lab", bufs=2))
            log_pool = ctx.enter_context(tc.tile_pool(name="log", bufs=2))
            exp_pool = ctx.enter_context(tc.tile_pool(name="expp", bufs=1))
            idp_pool = ctx.enter_context(tc.tile_pool(name="idpp", bufs=1))
            dig_pool = ctx.enter_context(tc.tile_pool(name="dig", bufs=2))
            stat_pool = ctx.enter_context(tc.tile_pool(name="stat", bufs=2))
            alo_pool = ctx.enter_context(tc.tile_pool(name="alo", bufs=1))
            acc_pool = ctx.enter_context(tc.tile_pool(name="acc", bufs=1))
            psum = ctx.enter_context(tc.tile_pool(name="psum", bufs=4, space="PSUM"))
            fin = ctx.enter_context(tc.tile_pool(name="fin", bufs=1))

            # ---- constants
            p_col = cpool.tile([P, 1], f32)
            nc.gpsimd.iota(p_col[:], pattern=[[0, 1]], base=0, channel_multiplier=1,
                           allow_small_or_imprecise_dtypes=True)
            ones_col = cpool.tile([P, 1], f32)
            nc.vector.memset(ones_col[:], 1.0)
            # c-row: column index replicated across partitions, fp16 (0..2047)
            crow = cpool.tile([P, W], f16)
            nc.gpsimd.iota(crow[:], pattern=[[1, W]], base=0, channel_multiplier=0,
                           allow_small_or_imprecise_dtypes=True)
            # bin id per finalize layout: bin = 128*k + p at [p, (img, s, k)]
            # (only used for exp accum staging / masks built later)

            exp_accs = acc_pool.tile([P, n_btiles], f32)

            accs = []
            for img in range(n_img):
                a = acc_pool.tile([M, NL], f32, tag=f"acc{img}")
                ay = acc_pool.tile([NH, NL], f32, tag=f"accy{img}")
                nc.vector.memset(a[:], 0.0)
                nc.vector.memset(ay[:], 0.0)
                accs.append((a, ay))

            for img in range(n_img):
                acc, accY = accs[img]
                for band in range(NB):
                    r0 = band * P
                    bt = img * NB + band
                    label_band = lab_pool.tile([P, W], i32, tag="label_band")
                    nc.gpsimd.dma_start(out=label_band[:], in_=label_h[img, r0:r0 + P, :])
                    logits_band = log_pool.tile([P, W], f32, tag="logits_band")
                    nc.scalar.dma_start(out=logits_band[:], in_=logits_h[img, r0:r0 + P, :])

                    # exp + per-partition row-sum on ACT (scratch out, 1-buf)
                    exp_scr = exp_pool.tile([P, W], f32, tag="exp_scr")
                    nc.scalar.activation(
                        out=exp_scr[:], in_=logits_band[:], func=Act.Exp,
                        accum_out=exp_accs[:, bt:bt + 1])

                    # id' = label - 101 in bf16 (exact: |id'| <= 256); invalid
                    # labels give negative id' -> negative hi -> no plane fires
                    idp_bf = idp_pool.tile([P, W], bf16, tag="idp")
                    nc.scalar.activation(out=idp_bf[:], in_=label_band[:],
                                         func=Act.Copy, bias=-101.0)
                    # hi = floor(id'/NL) via RNE(label*(1/NL) + bias) on ACT -> i32
                    hi_i = idp_pool.tile([P, W], i32, tag="hi_i32")
                    nc.vector.tensor_scalar(out=hi_i[:], in0=label_band[:],
                                            scalar1=-101.0 - (NL - 1) / 2.0,
                                            scalar2=1.0 / NL,
                                            op0=Alu.add, op1=Alu.mult)
                    hi_bf = dig_pool.tile([P, W], bf16, tag="hi_bf")
                    nc.scalar.activation(out=hi_bf[:], in_=hi_i[:], func=Act.Copy)
                    # lo = id' - NL*hi (bf16 exact)
                    lo_bf = dig_pool.tile([P, W], bf16, tag="lo_bf")
                    nc.vector.scalar_tensor_tensor(out=lo_bf[:], in0=hi_bf[:],
                                                   scalar=-float(NL), in1=idp_bf[:],
                                                   op0=Alu.mult, op1=Alu.add)

                    ps = psum.tile([M, NL], f32, tag="ps")
                    for ch in range(NCH):
                        c0 = ch * G
                        stat = stat_pool.tile([P, M * G], f16, tag="stat")
                        stat_v = stat[:].rearrange("p (m c) -> p m c", m=M)
                        alo = alo_pool.tile([P, NL * G], f16, tag="alo")
                        alo_v = alo[:].rearrange("p (l c) -> p l c", l=NL)

                        for h in range(NH):
                            # hi-onehot
                            nc.vector.tensor_scalar(
                                out=stat_v[:, h, :], in0=hi_bf[:, c0:c0 + G],
                                scalar1=float(h), scalar2=None, op0=Alu.is_equal)
                        # p * hi-onehot: first nb planes as one bulk ACT
                        # copy-scale, the rest fused on DVE
                        nb = NH if act_p < 0 else act_p
                        if nb > 0:
                            nc.scalar.activation(
                                out=stat_v[:, NH:NH + nb, :],
                                in_=stat_v[:, 0:nb, :], func=Act.Copy,
                                scale=p_col[:, 0:1])
                        for h in range(nb, NH):
                            nc.vector.tensor_scalar(
                                out=stat_v[:, NH + h, :],
                                in0=hi_bf[:, c0:c0 + G],
                                scalar1=float(h), scalar2=p_col[:, 0:1],
                                op0=Alu.is_equal, op1=Alu.mult)
                        # c * hi-onehot via tensor_tensor against c-row
                        for h in range(NH):
                            eng = nc.gpsimd if h < csplit else nc.vector
                            eng.tensor_tensor(
                                out=stat_v[:, 2 * NH + h, :],
                                in0=stat_v[:, h, :],
                                in1=crow[:, c0:c0 + G],
                                op=Alu.mult)
                        for l in range(NL):
                            nc.vector.tensor_scalar(
                                out=alo_v[:, l, :], in0=lo_bf[:, c0:c0 + G],
                                scalar1=float(l), scalar2=None, op0=Alu.is_equal)

                        for g in range(G):
                            nc.tensor.matmul(
                                out=ps[:],
                                lhsT=stat_v[:, :, g],
                                rhs=alo_v[:, :, g],
                                start=(ch == 0 and g == 0),
                                stop=(ch == NCH - 1 and g == G - 1),
                            )

                    # evacuate band: acc += ps; accY += 128*band*cnt
                    nc.vector.tensor_tensor(out=acc[:], in0=acc[:], in1=ps[:],
                                            op=Alu.add)
                    if band:
                        nc.vector.scalar_tensor_tensor(
                            out=accY[:], in0=ps[0:NH, :], scalar=float(P * band),
                            in1=accY[:], op0=Alu.mult, op1=Alu.add)

            # ---- bounce stats to DRAM and reload in finalize layout ----
            # bounce flat layout: img*1024 + s*256 + bin   (bin = 16*h + l)
            for img in range(n_img):
                acc, accY = accs[img]
                base = img * 4 * 256
                nc.gpsimd.dma_start(
                    out=bounce_h[base:base + 3 * 256].rearrange("(p c) -> p c", p=M),
                    in_=acc[:])
                nc.gpsimd.dma_start(
                    out=bounce_h[base + 3 * 256:base + 4 * 256]
                    .rearrange("(p c) -> p c", p=NH), in_=accY[:])

            # reload: t[p, (img, s2, k)] = bounce[img*1024 + s*256 + 128k + p]
            def reload(s):
                t = fin.tile([P, n_img * 2], f32, tag=f"re{s}")
                src = bounce_h[:].rearrange("(i s k p) -> p i s k", i=n_img, s=4, k=2)
                for img in range(n_img):
                    nc.gpsimd.dma_start(out=t[:, img * 2:(img + 1) * 2],
                                        in_=src[:, img, s, :])
                return t

            cnt = reload(0)      # counts
            syp = reload(1)      # sum of p (y-fine)
            sx = reload(2)       # sum of c (exact full x)
            ycrs = reload(3)     # 128*band-weighted counts (y-coarse)
            sy = fin.tile([P, n_img * 2], f32, tag="sy")
            nc.vector.tensor_tensor(out=sy[:], in0=syp[:], in1=ycrs[:], op=mybir.AluOpType.add)

            denom = fin.tile([P, n_img * 2], f32, tag="denom")
            nc.vector.tensor_scalar(out=denom[:], in0=cnt[:], scalar1=1.0, scalar2=None,
                                    op0=Alu.max)
            rcp = fin.tile([P, n_img * 2], f32, tag="rcp")
            nc.vector.reciprocal(rcp[:], denom[:])

            def floordiv(s_t, nm):
                # exact floor(s/denom): approx quotient then +/-1 fix
                qf = fin.tile([P, n_img * 2], f32, tag=f"qf{nm}")
                nc.vector.tensor_tensor(out=qf[:], in0=s_t[:], in1=rcp[:], op=Alu.mult)
                qi = fin.tile([P, n_img * 2], i32, tag=f"qi{nm}")
                nc.vector.tensor_copy(qi[:], qf[:])
                q = fin.tile([P, n_img * 2], f32, tag=f"q{nm}")
                nc.vector.tensor_copy(q[:], qi[:])
                r = fin.tile([P, n_img * 2], f32, tag=f"r{nm}")
                nc.vector.tensor_tensor(out=r[:], in0=q[:], in1=denom[:], op=Alu.mult)
                nc.vector.tensor_tensor(out=r[:], in0=s_t[:], in1=r[:], op=Alu.subtract)
                corr = fin.tile([P, n_img * 2], f32, tag=f"corr{nm}")
                nc.vector.tensor_tensor(out=corr[:], in0=r[:], in1=denom[:], op=Alu.is_ge)
                nc.vector.tensor_tensor(out=q[:], in0=q[:], in1=corr[:], op=Alu.add)
                nc.vector.tensor_scalar(out=corr[:], in0=r[:], scalar1=0.0, scalar2=None,
                                        op0=Alu.is_lt)
                nc.vector.tensor_tensor(out=q[:], in0=q[:], in1=corr[:], op=Alu.subtract)
                return q

            qy = floordiv(sy, "y")
            qx = floordiv(sx, "x")

            offs_f = fin.tile([P, n_img * 2], f32, tag="offs_f")
            nc.vector.scalar_tensor_tensor(out=offs_f[:], in0=qy[:], scalar=float(W),
                                           in1=qx[:], op0=Alu.mult, op1=Alu.add)
            # all bins are ids > 100; only mask = cnt > 0 (and bin 255 unused,
            # its cnt is 0). Also zero offsets for masked bins (safe gather).
            mask = fin.tile([P, n_img * 2], f32, tag="mask")
            nc.vector.tensor_scalar(out=mask[:], in0=cnt[:], scalar1=0.0, scalar2=None,
                                    op0=Alu.is_gt)
            nc.vector.tensor_tensor(out=offs_f[:], in0=offs_f[:], in1=mask[:], op=Alu.mult)
            offs_i = fin.tile([P, n_img * 2], i32, tag="offs_i")
            nc.vector.tensor_copy(offs_i[:], offs_f[:])

            gath = fin.tile([P, n_img * 2], f32, tag="gath")
            for img in range(n_img):
                for k in range(2):
                    col = img * 2 + k
                    nc.gpsimd.indirect_dma_start(
                        out=gath[:, col:col + 1],
                        out_offset=None,
                        in_=logits_h[:].rearrange("i h w -> (i h w)").unsqueeze(1),
                        in_offset=bass.IndirectOffsetOnAxis(
                            ap=offs_i[:, col:col + 1], axis=0),
                        element_offset=img * H * W,
                    )

            nc.vector.tensor_tensor(out=gath[:], in0=gath[:], in1=mask[:], op=Alu.mult)

            red = fin.tile([P, n_img + 1], f32, tag="red")
            for img in range(n_img):
                nc.vector.tensor_reduce(out=red[:, img:img + 1],
                                        in_=gath[:, img * 2:(img + 1) * 2],
                                        axis=mybir.AxisListType.X, op=Alu.add)
            nc.vector.tensor_reduce(out=red[:, n_img:n_img + 1], in_=exp_accs[:],
                                    axis=mybir.AxisListType.X, op=Alu.add)

            ps_fin = psum.tile([1, n_img + 1], f32, tag="ps_fin")
            nc.tensor.matmul(out=ps_fin[:], lhsT=ones_col[:], rhs=red[:],
                             start=True, stop=True)

            out_sb = fin.tile([1, 4], f32, tag="out_sb")
            nc.vector.memset(out_sb[:], 0.0)
            nc.vector.tensor_copy(out_sb[:, 0:1], ps_fin[:, n_img:n_img + 1])
            for img in range(n_img):
                nc.vector.tensor_copy(out_sb[:, 1 + img:2 + img], ps_fin[:, img:img + 1])
            nc.gpsimd.dma_start(out=out_h[:], in_=out_sb[:])

    nc.compile()
    return nc


_NC_CACHE = {}


def kernel(logits, label):
    logits = np.ascontiguousarray(np.asarray(logits, dtype=np.float32))
    label = np.ascontiguousarray(np.asarray(label, dtype=np.int32))
    assert logits.shape == (B, H, W), logits.shape
    assert label.shape == (B, H, W), label.shape

    from concourse.bass_utils import run_bass_kernel_spmd

    key = (NIMG, H, W)
    if key not in _NC_CACHE:
        _NC_CACHE[key] = _build_nc(NIMG, H, W)
    nc = _NC_CACHE[key]

    in_maps = [
        {"logits": logits[c * NIMG:(c + 1) * NIMG],
         "label": label[c * NIMG:(c + 1) * NIMG]}
        for c in range(N_CORES)
    ]
    import time as _time
    last_exc = None
    for attempt in range(4):
        try:
            res = run_bass_kernel_spmd(nc, in_maps, list(range(N_CORES)))
            break
        except Exception as e:
            last_exc = e
            _time.sleep(2.0 * (attempt + 1))
    else:
        raise last_exc

    exp_total = 0.0
    inst_total = 0.0
    for c in range(N_CORES):
        o = res.results[c]["out"][0]
        exp_total += float(o[0])
        for i in range(NIMG):
            inst_total += float(o[1 + i])
    int_loss = exp_total / float(B * H * W)
    inst = inst_total / float(B)
    return np.float32(int_loss - inst)


# revision 6
# speedup vs baseline: 1.2726x; 1.0022x over previous
"""Trainium2 Bass kernel v2 for nn_PoissonNLLLoss (B=16, H=1024, W=2048).

Computes  mean(exp(logits)) - mean_img( sum_{id>100,cnt>0} logits[cy,cx] )
with exact integer segment statistics, matching the jax reference.

v2 design (vs baseline):
  - 256 bins via id' = label - 101  (valid ids 101..355 -> 0..254; invalid
    labels give negative id' whose hi-digit never matches -> self-masking).
  - digits: hi = floor(id'/16) via an exact round-to-nearest trick on DVE
    (i32), cast to bf16 on ACT; lo = id' - 16*hi in bf16 (all values exact).
  - one-hot planes built with fused tensor_scalar ops (4x DVE mode):
      stat[h]      = (hi == h)                  [is_equal]
      stat[16+h]   = (hi == h) * p              [is_equal , mult p-col AP]
      stat[32+h]   = (hi == h) * c              [tensor_tensor vs c-row, 2x]
      alo[l]       = (lo == l)                  [is_equal]
    c-plane values are the FULL column index (0..2047) — exact in fp16 —
    so there are no octant corrections and a single PSUM accumulator per
    band with a 3-op evacuation.
  - matmul lhsT = stat[:, :, j] [128, 48] fp16, rhs = alo[:, :, j]
    [128, 16] fp16 -> psum [48, 16] accumulated over the whole band.
  - finalize: bounce stats to DRAM, reload as [128, 12], exact floor
    division, one indirect-DMA gather per (img, half), masked sum.
"""

import numpy as np

P = 128
NH = 16          # hi digit values
NL = 16          # lo digit values
NSTAT = 3        # {1, p, c}
M = NSTAT * NH   # 48 stationary rows

B, H, W = 16, 1024, 2048
N_CORES = 8
NIMG = B // N_CORES


def _build_nc(n_img, H, W, G=512, csplit=7, act_p=-1, NH=16, NL=16):
    # csplit: how many of the NH c-planes go to GPSIMD (rest on DVE)
    # act_p: how many of the NH p-planes go to ACT (copy-scale; rest DVE fused)
    import concourse.bass as bass
    import concourse.bacc as bacc
    import concourse.tile as tile
    from concourse import mybir

    f32 = mybir.dt.float32
    i32 = mybir.dt.int32
    bf16 = mybir.dt.bfloat16
    f16 = mybir.dt.float16
    Alu = mybir.AluOpType
    Act = mybir.ActivationFunctionType

    NB = H // P              # bands per image (8)
    NCH = W // G             # chunks per band
    n_btiles = n_img * NB
    M = NSTAT * NH           # stationary rows
    assert NH * NL == 256

    nc = bacc.Bacc('TRN2', target_bir_lowering=False, debug=False)
    logits_h = nc.declare_dram_parameter("logits", [n_img, H, W], f32, isOutput=False)
    label_h = nc.declare_dram_parameter("label", [n_img, H, W], i32, isOutput=False)
    out_h = nc.declare_dram_parameter("out", [1, 4], f32, isOutput=True)
    # bounce: per image: cnt[256], sy_p[256], sx[256], ycoarse[256]
    bounce_h = nc.dram_tensor("bounce", [n_img * 4 * 256], f32)

    with tile.TileContext(nc) as tc:
        import contextlib
        ctx = contextlib.ExitStack()
        with ctx:
            cpool = ctx.enter_context(tc.tile_pool(name="consts", bufs=1))
            lab_pool = ctx.enter_context(tc.tile_pool(name="lab", bufs=2))
            log_pool = ctx.enter_context(tc.tile_pool(name="log", bufs=2))
            exp_pool = ctx.enter_context(tc.tile_pool(name="expp", bufs=1))
            idp_pool = ctx.enter_context(tc.tile_pool(name="idpp", bufs=1))
            dig_pool = ctx.enter_context(tc.tile_pool(name="dig", bufs=2))
            stat_pool = ctx.enter_context(tc.tile_pool(name="stat", bufs=2))
            alo_pool = ctx.enter_context(tc.tile_pool(name="alo", bufs=1))
            acc_pool = ctx.enter_context(tc.tile_pool(name="acc", bufs=1))
            psum = ctx.enter_context(tc.tile_pool(name="psum", bufs=4, space="PSUM"))
            fin = ctx.enter_context(tc.tile_pool(name="fin", bufs=1))

            # ---- constants
            p_col = cpool.tile([P, 1], f32)
            nc.gpsimd.iota(p_col[:], pattern=[[0, 1]], base=0, channel_multiplier=1,
                           allow_small_or_imprecise_dtypes=True)
            ones_col = cpool.tile([P, 1], f32)
            nc.vector.memset(ones_col[:], 1.0)
            # c-row: column index replicated across partitions, fp16 (0..2047)
            crow = cpool.tile([P, W], f16)
            nc.gpsimd.iota(crow[:], pattern=[[1, W]], base=0, channel_multiplier=0,
                           allow_small_or_imprecise_dtypes=True)
            # bin id per finalize layout: bin = 128*k + p at [p, (img, s, k)]
            # (only used for exp accum staging / masks built later)

            exp_accs = acc_pool.tile([P, n_btiles], f32)

            accs = []
            for img in range(n_img):
                a = acc_pool.tile([M, NL], f32, tag=f"acc{img}")
                ay = acc_pool.tile([NH, NL], f32, tag=f"accy{img}")
                nc.vector.memset(a[:], 0.0)
                nc.vector.memset(ay[:], 0.0)
                accs.append((a, ay))

            for img in range(n_img):
                acc, accY = accs[img]
                for band in range(NB):
                    r0 = band * P
                    bt = img * NB + band
                    label_band = lab_pool.tile([P, W], i32, tag="label_band")
                    nc.gpsimd.dma_start(out=label_band[:], in_=label_h[img, r0:r0 + P, :])
                    logits_band = log_pool.tile([P, W], f32, tag="logits_band")
                    nc.scalar.dma_start(out=logits_band[:], in_=logits_h[img, r0:r0 + P, :])

                    # exp + per-partition row-sum on ACT (scratch out, 1-buf)
                    exp_scr = exp_pool.tile([P, W], f32, tag="exp_scr")
                    nc.scalar.activation(
                        out=exp_scr[:], in_=logits_band[:], func=Act.Exp,
                        accum_out=exp_accs[:, bt:bt + 1])

                    # id' = label - 101 in bf16 (exact: |id'| <= 256); invalid
                    # labels give negative id' -> negative hi -> no plane fires
                    idp_bf = idp_pool.tile([P, W], bf16, tag="idp")
                    nc.scalar.activation(out=idp_bf[:], in_=label_band[:],
                                         func=Act.Copy, bias=-101.0)
                    # hi = floor(id'/NL) via RNE(label*(1/NL) + bias) on ACT -> i32
                    hi_i = idp_pool.tile([P, W], i32, tag="hi_i32")
                    nc.vector.tensor_scalar(out=hi_i[:], in0=label_band[:],
                                            scalar1=-101.0 - (NL - 1) / 2.0,
                                            scalar2=1.0 / NL,
                                            op0=Alu.add, op1=Alu.mult)
                    hi_bf = dig_pool.tile([P, W], bf16, tag="hi_bf")
                    nc.scalar.activation(out=hi_bf[:], in_=hi_i[:], func=Act.Copy)
                    # lo = id' - NL*hi (bf16 exact)
                    lo_bf = dig_pool.tile([P, W], bf16, tag="lo_bf")
                    nc.vector.scalar_tensor_tensor(out=lo_bf[:], in0=hi_bf[:],
                                                   scalar=-float(NL), in1=idp_bf[:],
                                                   op0=Alu.mult, op1=Alu.add)

                    ps = psum.tile([M, NL], f32, tag="ps")
                    for ch in range(NCH):
                        c0 = ch * G
                        stat = stat_pool.tile([P, M * G], f16, tag="stat")
                        stat_v = stat[:].rearrange("p (m c) -> p m c", m=M)
                        alo = alo_pool.tile([P, NL * G], f16, tag="alo")
                        alo_v = alo[:].rearrange("p (l c) -> p l c", l=NL)

                        for h in range(NH):
                            # hi-onehot
                            nc.vector.tensor_scalar(
                                out=stat_v[:, h, :], in0=hi_bf[:, c0:c0 + G],
                                scalar1=float(h), scalar2=None, op0=Alu.is_equal)
                        # p * hi-onehot: first nb planes as one bulk ACT
                        # copy-scale, the rest fused on DVE
                        nb = NH if act_p < 0 else act_p
                        if nb > 0:
                            nc.scalar.activation(
                                out=stat_v[:, NH:NH + nb, :],
                                in_=stat_v[:, 0:nb, :], func=Act.Copy,
                                scale=p_col[:, 0:1])
                        for h in range(nb, NH):
                            nc.vector.tensor_scalar(
                                out=stat_v[:, NH + h, :],
                                in0=hi_bf[:, c0:c0 + G],
                                scalar1=float(h), scalar2=p_col[:, 0:1],
                                op0=Alu.is_equal, op1=Alu.mult)
                        # c * hi-onehot via tensor_tensor against c-row
                        for h in range(NH):
                            eng = nc.gpsimd if h < csplit else nc.vector
                            eng.tensor_tensor(
                                out=stat_v[:, 2 * NH + h, :],
                                in0=stat_v[:, h, :],
                                in1=crow[:, c0:c0 + G],
                                op=Alu.mult)
                        for l in range(NL):
                            nc.vector.tensor_scalar(
                                out=alo_v[:, l, :], in0=lo_bf[:, c0:c0 + G],
                                scalar1=float(l), scalar2=None, op0=Alu.is_equal)

                        for g in range(G):
                            nc.tensor.matmul(
                                out=ps[:],
                                lhsT=stat_v[:, :, g],
                                rhs=alo_v[:, :, g],
                                start=(ch == 0 and g == 0),
                                stop=(ch == NCH - 1 and g == G - 1),
                            )

                    # evacuate band: acc += ps; accY += 128*band*cnt
                    nc.vector.tensor_tensor(out=acc[:], in0=acc[:], in1=ps[:],
                                            op=Alu.add)
                    if band:
                        nc.vector.scalar_tensor_tensor(
                            out=accY[:], in0=ps[0:NH, :], scalar=float(P * band),
                            in1=accY[:], op0=Alu.mult, op1=Alu.add)

            # ---- bounce stats to DRAM and reload in finalize layout ----
            # bounce flat layout: img*1024 + s*256 + bin   (bin = 16*h + l)
            for img in range(n_img):
                acc, accY = accs[img]
                base = img * 4 * 256
                nc.gpsimd.dma_start(
                    out=bounce_h[base:base + 3 * 256].rearrange("(p c) -> p c", p=M),
                    in_=acc[:])
                nc.gpsimd.dma_start(
                    out=bounce_h[base + 3 * 256:base + 4 * 256]
                    .rearrange("(p c) -> p c", p=NH), in_=accY[:])

            # reload: t[p, (img, s2, k)] = bounce[img*1024 + s*256 + 128k + p]
            def reload(s):
                t = fin.tile([P, n_img * 2], f32, tag=f"re{s}")
                src = bounce_h[:].rearrange("(i s k p) -> p i s k", i=n_img, s=4, k=2)
                for img in range(n_img):
                    nc.gpsimd.dma_start(out=t[:, img * 2:(img + 1) * 2],
                                        in_=src[:, img, s, :])
                return t

            cnt = reload(0)      # counts
            syp = reload(1)      # sum of p (y-fine)
            sx = reload(2)       # sum of c (exact full x)
            ycrs = reload(3)     # 128*band-weighted counts (y-coarse)
            sy = fin.tile([P, n_img * 2], f32, tag="sy")
            nc.vector.tensor_tensor(out=sy[:], in0=syp[:], in1=ycrs[:], op=mybir.AluOpType.add)

            denom = fin.tile([P, n_img * 2], f32, tag="denom")
            nc.vector.tensor_scalar(out=denom[:], in0=cnt[:], scalar1=1.0, scalar2=None,
                                    op0=Alu.max)
            rcp = fin.tile([P, n_img * 2], f32, tag="rcp")
            nc.vector.reciprocal(rcp[:], denom[:])

            def floordiv(s_t, nm):
                # exact floor(s/denom): approx quotient then +/-1 fix
                qf = fin.tile([P, n_img * 2], f32, tag=f"qf{nm}")
                nc.vector.tensor_tensor(out=qf[:], in0=s_t[:], in1=rcp[:], op=Alu.mult)
                qi = fin.tile([P, n_img * 2], i32, tag=f"qi{nm}")
                nc.vector.tensor_copy(qi[:], qf[:])
                q = fin.tile([P, n_img * 2], f32, tag=f"q{nm}")
                nc.vector.tensor_copy(q[:], qi[:])
                r = fin.tile([P, n_img * 2], f32, tag=f"r{nm}")
                nc.vector.tensor_tensor(out=r[:], in0=q[:], in1=denom[:], op=Alu.mult)
                nc.vector.tensor_tensor(out=r[:], in0=s_t[:], in1=r[:], op=Alu.subtract)
                corr = fin.tile([P, n_img * 2], f32, tag=f"corr{nm}")
                nc.vector.tensor_tensor(out=corr[:], in0=r[:], in1=denom[:], op=Alu.is_ge)
                nc.vector.tensor_tensor(out=q[:], in0=q[:], in1=corr[:], op=Alu.add)
                nc.vector.tensor_scalar(out=corr[:], in0=r[:], scalar1=0.0, scalar2=None,
                                        op0=Alu.is_lt)
                nc.vector.tensor_tensor(out=q[:], in0=q[:], in1=corr[:], op=Alu.subtract)
                return q

            qy = floordiv(sy, "y")
            qx = floordiv(sx, "x")

            offs_f = fin.tile([P, n_img * 2], f32, tag="offs_f")
            nc.vector.scalar_tensor_tensor(out=offs_f[:], in0=qy[:], scalar=float(W),
                                           in1=qx[:], op0=Alu.mult, op1=Alu.add)
            # all bins are ids > 100; only mask = cnt > 0 (and bin 255 unused,
            # its cnt is 0). Also zero offsets for masked bins (safe gather).
            mask = fin.tile([P, n_img * 2], f32, tag="mask")
            nc.vector.tensor_scalar(out=mask[:], in0=cnt[:], scalar1=0.0, scalar2=None,
                                    op0=Alu.is_gt)
            nc.vector.tensor_tensor(out=offs_f[:], in0=offs_f[:], in1=mask[:], op=Alu.mult)
            offs_i = fin.tile([P, n_img * 2], i32, tag="offs_i")
            nc.vector.tensor_copy(offs_i[:], offs_f[:])

            gath = fin.tile([P, n_img * 2], f32, tag="gath")
            for img in range(n_img):
                for k in range(2):
                    col = img * 2 + k
                    nc.gpsimd.indirect_dma_start(
                        out=gath[:, col:col + 1],
                        out_offset=None,
                        in_=logits_h[:].rearrange("i h w -> (i h w)").unsqueeze(1),
                        in_offset=bass.IndirectOffsetOnAxis(
                            ap=offs_i[:, col:col + 1], axis=0),
                        element_offset=img * H * W,
                    )

            nc.vector.tensor_tensor(out=gath[:], in0=gath[:], in1=mask[:], op=Alu.mult)

            red = fin.tile([P, n_img + 1], f32, tag="red")
            for img in range(n_img):
                nc.vector.tensor_reduce(out=red[:, img:img + 1],
                                        in_=gath[:, img * 2:(img + 1) * 2],
                                        axis=mybir.AxisListType.X, op=Alu.add)
            nc.vector.tensor_reduce(out=red[:, n_img:n_img + 1], in_=exp_accs[:],
                                    axis=mybir.AxisListType.X, op=Alu.add)

            ps_fin = psum.tile([1, n_img + 1], f32, tag="ps_fin")
            nc.tensor.matmul(out=ps_fin[:], lhsT=ones_col[:], rhs=red[:],
                             start=True, stop=True)

            out_sb = fin.tile([1, 4], f32, tag="out_sb")
            nc.vector.memset(out_sb[:], 0.0)
            nc.vector.tensor_copy(out_sb[:, 0:1], ps_fin[:, n_img:n_img + 1])
            for img in range(n_img):
                nc.vector.tensor_copy(out_sb[:, 1 + img:2 + img], ps_fin[:, img:img + 1])
            nc.gpsimd.dma_start(out=out_h[:], in_=out_sb[:])

    nc.compile()
    return nc


_NC_CACHE = {}


def kernel(logits, label):
    logits = np.ascontiguousarray(np.asarray(logits, dtype=np.float32))
    label = np.ascontiguousarray(np.asarray(label, dtype=np.int32))
    assert logits.shape == (B, H, W), logits.shape
    assert label.shape == (B, H, W), label.shape

    from concourse.bass_utils import run_bass_kernel_spmd

    key = (NIMG, H, W)
    if key not in _NC_CACHE:
        _NC_CACHE[key] = _build_nc(NIMG, H, W)
    nc = _NC_CACHE[key]

    in_maps = [
        {"logits": logits[c * NIMG:(c + 1) * NIMG],
         "label": label[c * NIMG:(c + 1) * NIMG]}
        for c in range(N_CORES)
    ]
    import time as _time
    last_exc = None
    for attempt in range(4):
        try:
            res = run_bass_kernel_spmd(nc, in_maps, list(range(N_CORES)))
            break
        except Exception as e:
            last_exc = e
            _time.sleep(2.0 * (attempt + 1))
    else:
        raise last_exc

    exp_total = 0.0
    inst_total = 0.0
    for c in range(N_CORES):
        o = res.results[c]["out"][0]
        exp_total += float(o[0])
        for i in range(NIMG):
            inst_total += float(o[1 + i])
    int_loss = exp_total / float(B * H * W)
    inst = inst_total / float(B)
    return np.float32(int_loss - inst)


# revision 7
# speedup vs baseline: 1.2813x; 1.0069x over previous
"""Trainium2 Bass kernel v2 for nn_PoissonNLLLoss (B=16, H=1024, W=2048).

Computes  mean(exp(logits)) - mean_img( sum_{id>100,cnt>0} logits[cy,cx] )
with exact integer segment statistics, matching the jax reference.

v2 design (vs baseline):
  - 256 bins via id' = label - 101  (valid ids 101..355 -> 0..254; invalid
    labels give negative id' whose hi-digit never matches -> self-masking).
  - digits: hi = floor(id'/16) via an exact round-to-nearest trick on DVE
    (i32), cast to bf16 on ACT; lo = id' - 16*hi in bf16 (all values exact).
  - one-hot planes built with fused tensor_scalar ops (4x DVE mode):
      stat[h]      = (hi == h)                  [is_equal]
      stat[16+h]   = (hi == h) * p              [is_equal , mult p-col AP]
      stat[32+h]   = (hi == h) * c              [tensor_tensor vs c-row, 2x]
      alo[l]       = (lo == l)                  [is_equal]
    c-plane values are the FULL column index (0..2047) — exact in fp16 —
    so there are no octant corrections and a single PSUM accumulator per
    band with a 3-op evacuation.
  - matmul lhsT = stat[:, :, j] [128, 48] fp16, rhs = alo[:, :, j]
    [128, 16] fp16 -> psum [48, 16] accumulated over the whole band.
  - finalize: bounce stats to DRAM, reload as [128, 12], exact floor
    division, one indirect-DMA gather per (img, half), masked sum.
"""

import numpy as np

P = 128
NH = 16          # hi digit values
NL = 16          # lo digit values
NSTAT = 3        # {1, p, c}
M = NSTAT * NH   # 48 stationary rows

B, H, W = 16, 1024, 2048
N_CORES = 8
NIMG = B // N_CORES


def _build_nc(n_img, H, W, G=512, csplit=7, act_p=-1, NH=16, NL=16):
    # csplit: how many of the NH c-planes go to GPSIMD (rest on DVE)
    # act_p: how many of the NH p-planes go to ACT (copy-scale; rest DVE fused)
    import concourse.bass as bass
    import concourse.bacc as bacc
    import concourse.tile as tile
    from concourse import mybir

    f32 = mybir.dt.float32
    i32 = mybir.dt.int32
    bf16 = mybir.dt.bfloat16
    f16 = mybir.dt.float16
    Alu = mybir.AluOpType
    Act = mybir.ActivationFunctionType

    NB = H // P              # bands per image (8)
    NCH = W // G             # chunks per band
    n_btiles = n_img * NB
    M = NSTAT * NH           # stationary rows
    assert NH * NL == 256

    nc = bacc.Bacc('TRN2', target_bir_lowering=False, debug=False)
    logits_h = nc.declare_dram_parameter("logits", [n_img, H, W], f32, isOutput=False)
    label_h = nc.declare_dram_parameter("label", [n_img, H, W], i32, isOutput=False)
    out_h = nc.declare_dram_parameter("out", [1, 4], f32, isOutput=True)
    # bounce: per image: cnt[256], sy_p[256], sx[256], ycoarse[256]
    bounce_h = nc.dram_tensor("bounce", [n_img * 4 * 256], f32)

    with tile.TileContext(nc) as tc:
        import contextlib
        ctx = contextlib.ExitStack()
        with ctx:
            cpool = ctx.enter_context(tc.tile_pool(name="consts", bufs=1))
            lab_pool = ctx.enter_context(tc.tile_pool(name="lab", bufs=2))
            log_pool = ctx.enter_context(tc.tile_pool(name="log", bufs=2))
            exp_pool = ctx.enter_context(tc.tile_pool(name="expp", bufs=1))
            idp_pool = ctx.enter_context(tc.tile_pool(name="idpp", bufs=1))
            dig_pool = ctx.enter_context(tc.tile_pool(name="dig", bufs=2))
            stat_pool = ctx.enter_context(tc.tile_pool(name="stat", bufs=2))
            alo_pool = ctx.enter_context(tc.tile_pool(name="alo", bufs=1))
            acc_pool = ctx.enter_context(tc.tile_pool(name="acc", bufs=1))
            psum = ctx.enter_context(tc.tile_pool(name="psum", bufs=4, space="PSUM"))
            fin = ctx.enter_context(tc.tile_pool(name="fin", bufs=1))

            # ---- constants
            p_col = cpool.tile([P, 1], f32)
            nc.gpsimd.iota(p_col[:], pattern=[[0, 1]], base=0, channel_multiplier=1,
                           allow_small_or_imprecise_dtypes=True)
            ones_col = cpool.tile([P, 1], f32)
            nc.vector.memset(ones_col[:], 1.0)
            # c-row: column index replicated across partitions, fp16 (0..2047)
            crow = cpool.tile([P, W], f16)
            nc.gpsimd.iota(crow[:], pattern=[[1, W]], base=0, channel_multiplier=0,
                           allow_small_or_imprecise_dtypes=True)
            # bin id per finalize layout: bin = 128*k + p at [p, (img, s, k)]
            # (only used for exp accum staging / masks built later)

            exp_accs = acc_pool.tile([P, n_btiles], f32)

            accs = []
            for img in range(n_img):
                a = acc_pool.tile([M, NL], f32, tag=f"acc{img}")
                ay = acc_pool.tile([NH, NL], f32, tag=f"accy{img}")
                nc.vector.memset(a[:], 0.0)
                nc.vector.memset(ay[:], 0.0)
                accs.append((a, ay))

            for img in range(n_img):
                acc, accY = accs[img]
                for band in range(NB):
                    r0 = band * P
                    bt = img * NB + band
                    label_band = lab_pool.tile([P, W], i32, tag="label_band")
                    nc.gpsimd.dma_start(out=label_band[:], in_=label_h[img, r0:r0 + P, :])
                    logits_band = log_pool.tile([P, W], f32, tag="logits_band")
                    nc.scalar.dma_start(out=logits_band[:], in_=logits_h[img, r0:r0 + P, :])

                    # exp + per-partition row-sum on ACT (scratch out, 1-buf)
                    exp_scr = exp_pool.tile([P, W], f32, tag="exp_scr")
                    nc.scalar.activation(
                        out=exp_scr[:], in_=logits_band[:], func=Act.Exp,
                        accum_out=exp_accs[:, bt:bt + 1])

                    # id' = label - 101 in bf16 (exact: |id'| <= 256); invalid
                    # labels give negative id' -> negative hi -> no plane fires
                    idp_bf = idp_pool.tile([P, W], bf16, tag="idp")
                    nc.scalar.activation(out=idp_bf[:], in_=label_band[:],
                                         func=Act.Copy, bias=-101.0)
                    # hi = floor(id'/NL) via RNE(label*(1/NL) + bias) on ACT -> i32
                    hi_i = idp_pool.tile([P, W], i32, tag="hi_i32")
                    nc.vector.tensor_scalar(out=hi_i[:], in0=label_band[:],
                                            scalar1=-101.0 - (NL - 1) / 2.0,
                                            scalar2=1.0 / NL,
                                            op0=Alu.add, op1=Alu.mult)
                    hi_bf = dig_pool.tile([P, W], bf16, tag="hi_bf")
                    nc.scalar.activation(out=hi_bf[:], in_=hi_i[:], func=Act.Copy)
                    # lo = id' - NL*hi (bf16 exact)
                    lo_bf = dig_pool.tile([P, W], bf16, tag="lo_bf")
                    nc.vector.scalar_tensor_tensor(out=lo_bf[:], in0=hi_bf[:],
                                                   scalar=-float(NL), in1=idp_bf[:],
                                                   op0=Alu.mult, op1=Alu.add)

                    ps = psum.tile([M, NL], f32, tag="ps")
                    for ch in range(NCH):
                        c0 = ch * G
                        stat = stat_pool.tile([P, M * G], f16, tag="stat")
                        stat_v = stat[:].rearrange("p (m c) -> p m c", m=M)
                        alo = alo_pool.tile([P, NL * G], f16, tag="alo")
                        alo_v = alo[:].rearrange("p (l c) -> p l c", l=NL)

                        for h in range(NH):
                            # hi-onehot
                            nc.vector.tensor_scalar(
                                out=stat_v[:, h, :], in0=hi_bf[:, c0:c0 + G],
                                scalar1=float(h), scalar2=None, op0=Alu.is_equal)
                        # p * hi-onehot: first nb planes as one bulk ACT
                        # copy-scale, the rest fused on DVE
                        nb = NH if act_p < 0 else act_p
                        if nb > 0:
                            nc.scalar.activation(
                                out=stat_v[:, NH:NH + nb, :],
                                in_=stat_v[:, 0:nb, :], func=Act.Copy,
                                scale=p_col[:, 0:1])
                        for h in range(nb, NH):
                            nc.vector.tensor_scalar(
                                out=stat_v[:, NH + h, :],
                                in0=hi_bf[:, c0:c0 + G],
                                scalar1=float(h), scalar2=p_col[:, 0:1],
                                op0=Alu.is_equal, op1=Alu.mult)
                        # c * hi-onehot via tensor_tensor against c-row
                        for h in range(NH):
                            eng = nc.gpsimd if h < csplit else nc.vector
                            eng.tensor_tensor(
                                out=stat_v[:, 2 * NH + h, :],
                                in0=stat_v[:, h, :],
                                in1=crow[:, c0:c0 + G],
                                op=Alu.mult)
                        for l in range(NL):
                            eng2 = nc.gpsimd if l < 1 else nc.vector
                            eng2.tensor_scalar(
                                out=alo_v[:, l, :], in0=lo_bf[:, c0:c0 + G],
                                scalar1=float(l), scalar2=None, op0=Alu.is_equal)

                        for g in range(G):
                            nc.tensor.matmul(
                                out=ps[:],
                                lhsT=stat_v[:, :, g],
                                rhs=alo_v[:, :, g],
                                start=(ch == 0 and g == 0),
                                stop=(ch == NCH - 1 and g == G - 1),
                            )

                    # evacuate band: acc += ps; accY += 128*band*cnt
                    nc.vector.tensor_tensor(out=acc[:], in0=acc[:], in1=ps[:],
                                            op=Alu.add)
                    if band:
                        nc.vector.scalar_tensor_tensor(
                            out=accY[:], in0=ps[0:NH, :], scalar=float(P * band),
                            in1=accY[:], op0=Alu.mult, op1=Alu.add)

            # ---- bounce stats to DRAM and reload in finalize layout ----
            # bounce flat layout: img*1024 + s*256 + bin   (bin = 16*h + l)
            for img in range(n_img):
                acc, accY = accs[img]
                base = img * 4 * 256
                nc.gpsimd.dma_start(
                    out=bounce_h[base:base + 3 * 256].rearrange("(p c) -> p c", p=M),
                    in_=acc[:])
                nc.gpsimd.dma_start(
                    out=bounce_h[base + 3 * 256:base + 4 * 256]
                    .rearrange("(p c) -> p c", p=NH), in_=accY[:])

            # reload: t[p, (img, s2, k)] = bounce[img*1024 + s*256 + 128k + p]
            def reload(s):
                t = fin.tile([P, n_img * 2], f32, tag=f"re{s}")
                src = bounce_h[:].rearrange("(i s k p) -> p i s k", i=n_img, s=4, k=2)
                for img in range(n_img):
                    nc.gpsimd.dma_start(out=t[:, img * 2:(img + 1) * 2],
                                        in_=src[:, img, s, :])
                return t

            cnt = reload(0)      # counts
            syp = reload(1)      # sum of p (y-fine)
            sx = reload(2)       # sum of c (exact full x)
            ycrs = reload(3)     # 128*band-weighted counts (y-coarse)
            sy = fin.tile([P, n_img * 2], f32, tag="sy")
            nc.vector.tensor_tensor(out=sy[:], in0=syp[:], in1=ycrs[:], op=mybir.AluOpType.add)

            denom = fin.tile([P, n_img * 2], f32, tag="denom")
            nc.vector.tensor_scalar(out=denom[:], in0=cnt[:], scalar1=1.0, scalar2=None,
                                    op0=Alu.max)
            rcp = fin.tile([P, n_img * 2], f32, tag="rcp")
            nc.vector.reciprocal(rcp[:], denom[:])

            def floordiv(s_t, nm):
                # exact floor(s/denom): approx quotient then +/-1 fix
                qf = fin.tile([P, n_img * 2], f32, tag=f"qf{nm}")
                nc.vector.tensor_tensor(out=qf[:], in0=s_t[:], in1=rcp[:], op=Alu.mult)
                qi = fin.tile([P, n_img * 2], i32, tag=f"qi{nm}")
                nc.vector.tensor_copy(qi[:], qf[:])
                q = fin.tile([P, n_img * 2], f32, tag=f"q{nm}")
                nc.vector.tensor_copy(q[:], qi[:])
                r = fin.tile([P, n_img * 2], f32, tag=f"r{nm}")
                nc.vector.tensor_tensor(out=r[:], in0=q[:], in1=denom[:], op=Alu.mult)
                nc.vector.tensor_tensor(out=r[:], in0=s_t[:], in1=r[:], op=Alu.subtract)
                corr = fin.tile([P, n_img * 2], f32, tag=f"corr{nm}")
                nc.vector.tensor_tensor(out=corr[:], in0=r[:], in1=denom[:], op=Alu.is_ge)
                nc.vector.tensor_tensor(out=q[:], in0=q[:], in1=corr[:], op=Alu.add)
                nc.vector.tensor_scalar(out=corr[:], in0=r[:], scalar1=0.0, scalar2=None,
                                        op0=Alu.is_lt)
                nc.vector.tensor_tensor(out=q[:], in0=q[:], in1=corr[:], op=Alu.subtract)
                return q

            qy = floordiv(sy, "y")
            qx = floordiv(sx, "x")

            offs_f = fin.tile([P, n_img * 2], f32, tag="offs_f")
            nc.vector.scalar_tensor_tensor(out=offs_f[:], in0=qy[:], scalar=float(W),
                                           in1=qx[:], op0=Alu.mult, op1=Alu.add)
            # all bins are ids > 100; only mask = cnt > 0 (and bin 255 unused,
            # its cnt is 0). Also zero offsets for masked bins (safe gather).
            mask = fin.tile([P, n_img * 2], f32, tag="mask")
            nc.vector.tensor_scalar(out=mask[:], in0=cnt[:], scalar1=0.0, scalar2=None,
                                    op0=Alu.is_gt)
            nc.vector.tensor_tensor(out=offs_f[:], in0=offs_f[:], in1=mask[:], op=Alu.mult)
            offs_i = fin.tile([P, n_img * 2], i32, tag="offs_i")
            nc.vector.tensor_copy(offs_i[:], offs_f[:])

            gath = fin.tile([P, n_img * 2], f32, tag="gath")
            for img in range(n_img):
                for k in range(2):
                    col = img * 2 + k
                    nc.gpsimd.indirect_dma_start(
                        out=gath[:, col:col + 1],
                        out_offset=None,
                        in_=logits_h[:].rearrange("i h w -> (i h w)").unsqueeze(1),
                        in_offset=bass.IndirectOffsetOnAxis(
                            ap=offs_i[:, col:col + 1], axis=0),
                        element_offset=img * H * W,
                    )

            nc.vector.tensor_tensor(out=gath[:], in0=gath[:], in1=mask[:], op=Alu.mult)

            red = fin.tile([P, n_img + 1], f32, tag="red")
            for img in range(n_img):
                nc.vector.tensor_reduce(out=red[:, img:img + 1],
                                        in_=gath[:, img * 2:(img + 1) * 2],
                                        axis=mybir.AxisListType.X, op=Alu.add)
            nc.vector.tensor_reduce(out=red[:, n_img:n_img + 1], in_=exp_accs[:],
                                    axis=mybir.AxisListType.X, op=Alu.add)

            ps_fin = psum.tile([1, n_img + 1], f32, tag="ps_fin")
            nc.tensor.matmul(out=ps_fin[:], lhsT=ones_col[:], rhs=red[:],
                             start=True, stop=True)

            out_sb = fin.tile([1, 4], f32, tag="out_sb")
            nc.vector.memset(out_sb[:], 0.0)
            nc.vector.tensor_copy(out_sb[:, 0:1], ps_fin[:, n_img:n_img + 1])
            for img in range(n_img):
                nc.vector.tensor_copy(out_sb[:, 1 + img:2 + img], ps_fin[:, img:img + 1])
            nc.gpsimd.dma_start(out=out_h[:], in_=out_sb[:])

    nc.compile()
    return nc


_NC_CACHE = {}


def kernel(logits, label):
    logits = np.ascontiguousarray(np.asarray(logits, dtype=np.float32))
    label = np.ascontiguousarray(np.asarray(label, dtype=np.int32))
    assert logits.shape == (B, H, W), logits.shape
    assert label.shape == (B, H, W), label.shape

    from concourse.bass_utils import run_bass_kernel_spmd

    key = (NIMG, H, W)
    if key not in _NC_CACHE:
        _NC_CACHE[key] = _build_nc(NIMG, H, W)
    nc = _NC_CACHE[key]

    in_maps = [
        {"logits": logits[c * NIMG:(c + 1) * NIMG],
         "label": label[c * NIMG:(c + 1) * NIMG]}
        for c in range(N_CORES)
    ]
    import time as _time
    last_exc = None
    for attempt in range(4):
        try:
            res = run_bass_kernel_spmd(nc, in_maps, list(range(N_CORES)))
            break
        except Exception as e:
            last_exc = e
            _time.sleep(2.0 * (attempt + 1))
    else:
        raise last_exc

    exp_total = 0.0
    inst_total = 0.0
    for c in range(N_CORES):
        o = res.results[c]["out"][0]
        exp_total += float(o[0])
        for i in range(NIMG):
            inst_total += float(o[1 + i])
    int_loss = exp_total / float(B * H * W)
    inst = inst_total / float(B)
    return np.float32(int_loss - inst)
